# revision 3
# baseline (speedup 1.0000x reference)
"""Trainium2 Bass kernel for nn_AttentionLayer (B=32, T=2048, D=512).

Computation (per batch b):
    s1 = x0 @ W_a                       # (D,)
    s2 = x1[b] @ W_h                    # (T, D)
    s  = tanh(s1 + s2)                  # (T, D)
    o  = V_a @ s                        # (D,)   (contract T)
    alpha = softmax(o)                  # over D
    out[b] = alpha * sum(x1[b])

Sharding: data-parallel over batch across 8 NeuronCores (4 batches/core),
weights replicated, no cross-core communication.

Per-core dataflow, default mode "bf16" (measured 128 us on TRN2, rel err
~2e-3; mode "f32" is the full-precision fallback at ~150 us, rel err ~5e-5):
  - SWDGE cast-DMA x1 natural tile [128t, 4s, 512d] fp32->bf16 into SBUF
  - x1 -> x1T chunks [128d, 512t] via REGULAR bf16 matmuls against an
    identity (1 cyc/row, keeps the PE HAM warm; transpose-mode measures
    ~350 ns/op and regular fp32 matmuls are 4 cyc/row)
  - PSUM->SBUF copies of x1T split across ACT/DVE; their accum_out emits
    per-partition partial sums of x1 for free (-> sum(x1[b]) later)
  - PE matmul (bf16): s2T[dout, t] = sum_k W_h[k, dout] * x1T[k, t]
  - ACT: tanh(s2T + bias s1T[dout, 1]) fused, PSUM -> SBUF
  - DVE scalar_tensor_tensor with partition-broadcast V_a: weighted t-sums
    (InstTensorTensorReduce and all-bf16 ScalarTensorTensor+accum both
    wedge TRN2; fp32 ScalarTensorTensor is the working form)
  - Epilogue: softmax over D on [4, 512] rows, scale by sum(x1[b])
"""

import numpy as np

B, T, D = 32, 2048, 512
NCORES = 8
BL = B // NCORES          # batches per core
P = 128
KC = D // P               # k chunks (4)
NCH = D // P              # output-d chunks (4)
TBLK = 512                # t elements per block
NTB = T // TBLK           # t blocks (4)
SUB = TBLK // P           # natural subtiles per block (4)


def build_nc(mode: str = "f32"):
    """Build the per-core Bass program. mode in {"f32", "bf16"}."""
    import concourse.bass as bass
    import concourse.tile as tile
    from concourse import bacc, mybir
    from concourse.masks import make_identity

    f32 = mybir.dt.float32
    f32r = mybir.dt.float32r
    bf16 = mybir.dt.bfloat16
    dt_nat = f32 if mode == "f32" else bf16     # natural x1 tiles
    dt_mm = f32r if mode == "f32" else bf16     # matmul operand tiles
    # tanh output / va / scr stay fp32: the bf16 ScalarTensorTensor+accum
    # path wedges TRN2 hardware (perf-mode + accumulator readout)
    dt_s = f32

    nc = bacc.Bacc("TRN2", target_bir_lowering=False)

    x0s = nc.dram_tensor("x0s", [BL, D], f32, kind="ExternalInput").ap()
    x1s = nc.dram_tensor("x1s", [BL, T, D], f32, kind="ExternalInput").ap()
    wa = nc.dram_tensor("W_a", [D, D], f32, kind="ExternalInput").ap()
    wh = nc.dram_tensor("W_h", [D, D], f32, kind="ExternalInput").ap()
    va = nc.dram_tensor("V_a", [1, T], f32, kind="ExternalInput").ap()
    out = nc.dram_tensor("out", [BL, D], f32, kind="ExternalOutput").ap()

    with tile.TileContext(nc) as tc:
        with (
            tc.tile_pool(name="consts", bufs=1) as consts,
            tc.tile_pool(name="nat", bufs=3) as nat_pool,
            tc.tile_pool(name="xt", bufs=8) as xt_pool,
            tc.tile_pool(name="s", bufs=4) as s_pool,
            tc.tile_pool(name="scr", bufs=2) as scr_pool,
            tc.tile_pool(name="small", bufs=1) as small,
            tc.tile_pool(name="misc_ps", bufs=1, space="PSUM") as misc_ps,
            tc.tile_pool(name="tp_ps", bufs=2, space="PSUM") as tp_pool,
            tc.tile_pool(name="mm_ps", bufs=4, space="PSUM") as mm_pool,
        ):
            # ---------------- constants ----------------
            ident = consts.tile([P, P], dt_nat, tag="ident")
            make_identity(nc, ident)
            identf = ident
            if dt_nat != f32:
                identf = consts.tile([P, P], f32, tag="identf")
                make_identity(nc, identf)

            # W_h in matmul dtype (f32r needs an explicit rounding producer)
            wh_sb = consts.tile([P, KC, D], dt_mm, tag="wh")
            if mode == "f32":
                wh_f32 = consts.tile([P, KC, D], f32, tag="wh_f32")
                nc.sync.dma_start(
                    out=wh_f32, in_=wh.rearrange("(c p) n -> p c n", p=P)
                )
                nc.vector.tensor_copy(out=wh_sb, in_=wh_f32)
            else:
                nc.gpsimd.dma_start(
                    out=wh_sb, in_=wh.rearrange("(c p) n -> p c n", p=P)
                )
            wa_sb = consts.tile([P, KC, D], f32, tag="wa")
            nc.sync.dma_start(out=wa_sb, in_=wa.rearrange("(c p) n -> p c n", p=P))

            va_sb = consts.tile([P, T], dt_s, tag="va")
            va_bcast = bass.AP(
                tensor=va.tensor, offset=va.offset, ap=[[0, P], va.ap[-1]]
            )
            nc.gpsimd.dma_start(out=va_sb, in_=va_bcast)

            # all-ones column for the final cross-partition sum
            ones_col = consts.tile([P, 1], f32, tag="ones_col")
            nc.vector.memset(ones_col, 1.0)

            # ---------------- phase 0: s1T = (x0 @ W_a)^T ----------------
            # pad x0 to 128 partitions: contraction dims < 128 are flaky on HW
            x0_nat = small.tile([P, D], f32, tag="x0_nat")
            nc.vector.memset(x0_nat, 0.0)
            nc.sync.dma_start(out=x0_nat[:BL, :], in_=x0s)
            x0t_sb = small.tile([P, KC, BL], f32, tag="x0t")
            for k in range(KC):
                x0t_ps = misc_ps.tile([P, P], f32, tag="misc")
                nc.tensor.transpose(
                    x0t_ps, x0_nat[:, k * P : (k + 1) * P], identf
                )
                nc.vector.tensor_copy(out=x0t_sb[:, k, :], in_=x0t_ps[:, :BL])

            s1t_sb = small.tile([P, NCH, BL], f32, tag="s1t")
            for n in range(NCH):
                s1_ps = misc_ps.tile([P, BL], f32, tag="misc")
                for k in range(KC):
                    nc.tensor.matmul(
                        s1_ps,
                        lhsT=wa_sb[:, k, n * P : (n + 1) * P],
                        rhs=x0t_sb[:, k, :],
                        start=(k == 0),
                        stop=(k == KC - 1),
                    )
                nc.vector.tensor_copy(out=s1t_sb[:, n, :], in_=s1_ps)

            # ---------------- main loop ----------------
            o_parts = small.tile([P, NCH * BL * NTB], f32, tag="o_parts")
            # per-(b,tb,dk) partial sums of x1 (per d-partition), free via the
            # accum_out of the transpose-copy ops
            xsums = small.tile([P, BL * NTB * KC], f32, tag="xsums")

            for b in range(BL):
                for tb in range(NTB):
                    nat = nat_pool.tile([P, SUB, D], dt_nat, tag="nat")
                    src = x1s[b, tb * TBLK : (tb + 1) * TBLK, :].rearrange(
                        "(s p) d -> p s d", p=P
                    )
                    if dt_nat == f32:
                        nc.sync.dma_start(out=nat, in_=src)
                    else:
                        nc.gpsimd.dma_start(out=nat, in_=src)  # SWDGE cast

                    # transpose x1 tile: [t, d] -> 4 chunks [128d, 512t];
                    # the PSUM->SBUF copies also emit per-partition sums.
                    # bf16: regular matmuls vs identity (1 cyc/row, keeps HAM
                    # warm) into fp32 PSUM; f32: transpose-mode.
                    xts = []
                    for dk in range(KC):
                        if dt_nat == bf16:
                            tp_ps = tp_pool.tile([P, TBLK], f32, tag="tp")
                            for s in range(SUB):
                                nc.tensor.matmul(
                                    tp_ps[:, s * P : (s + 1) * P],
                                    lhsT=nat[:, s, dk * P : (dk + 1) * P],
                                    rhs=ident,
                                    start=True,
                                    stop=True,
                                )
                        else:
                            tp_ps = tp_pool.tile([P, TBLK], dt_nat, tag="tp")
                            for s in range(SUB):
                                nc.tensor.transpose(
                                    tp_ps[:, s * P : (s + 1) * P],
                                    nat[:, s, dk * P : (dk + 1) * P],
                                    ident,
                                )
                        xt = xt_pool.tile([P, TBLK], dt_mm, tag="xt")
                        xi = (b * NTB + tb) * KC + dk
                        if dk < 2:
                            nc.scalar.activation(
                                out=xt,
                                in_=tp_ps,
                                func=mybir.ActivationFunctionType.Copy,
                                accum_out=xsums[:, xi : xi + 1],
                            )
                        else:
                            nc.vector.tensor_scalar(
                                out=xt,
                                in0=tp_ps,
                                scalar1=0.0,
                                scalar2=0.0,
                                op0=mybir.AluOpType.add,
                                op1=mybir.AluOpType.add,
                                accum_out=xsums[:, xi : xi + 1],
                            )
                        xts.append(xt)

                    # s2T = W_h^T x1^T ; tanh(+s1T bias) ; V_a-weighted t-sum
                    for n in range(NCH):
                        mm_ps = mm_pool.tile([P, TBLK], f32, tag="mm")
                        for dk in range(KC):
                            nc.tensor.matmul(
                                mm_ps,
                                lhsT=wh_sb[:, dk, n * P : (n + 1) * P],
                                rhs=xts[dk][:],
                                start=(dk == 0),
                                stop=(dk == KC - 1),
                            )
                        s_sb = s_pool.tile([P, TBLK], dt_s, tag="s")
                        nc.scalar.activation(
                            out=s_sb,
                            in_=mm_ps,
                            func=mybir.ActivationFunctionType.Tanh,
                            bias=s1t_sb[:, n, b : b + 1],
                        )
                        scr = scr_pool.tile([P, TBLK], dt_s, tag="scr")
                        idx = (n * BL + b) * NTB + tb
                        # out = (s * 1.0) * va ; accum_out = sum(out) per
                        # partition. (InstTensorTensorReduce wedges TRN2 here;
                        # InstScalarTensorTensor is the working equivalent.)
                        nc.vector.scalar_tensor_tensor(
                            out=scr,
                            in0=s_sb,
                            scalar=1.0,
                            in1=va_sb[:, tb * TBLK : (tb + 1) * TBLK],
                            op0=mybir.AluOpType.mult,
                            op1=mybir.AluOpType.mult,
                            accum_out=o_parts[:, idx : idx + 1],
                        )

            # ---------------- epilogue ----------------
            # o (transposed layout) = sum of partials over t-blocks
            o_sb = small.tile([P, NCH * BL], f32, tag="o_sb")
            nc.vector.reduce_sum(
                out=o_sb,
                in_=o_parts.rearrange("p (q t) -> p q t", t=NTB),
                axis=mybir.AxisListType.X,
            )
            # transpose o -> [b, d]
            o3 = o_sb.rearrange("p (n b) -> p n b", b=BL)
            ot_ps = misc_ps.tile([BL, D], f32, tag="misc")
            for n in range(NCH):
                nc.tensor.transpose(
                    ot_ps[:, n * P : (n + 1) * P], o3[:, n, :], identf
                )
            ot_sb = small.tile([BL, D], f32, tag="ot_sb")
            nc.vector.tensor_copy(out=ot_sb, in_=ot_ps)

            # sum(x1[b]) finish: reduce per-batch columns, then contract the
            # partition (d) axis with a ones-vector matmul -> [BL, 1]
            xb_sb = small.tile([P, BL], f32, tag="xb_sb")
            nc.vector.reduce_sum(
                out=xb_sb,
                in_=xsums.rearrange("p (b q) -> p b q", q=NTB * KC),
                axis=mybir.AxisListType.X,
            )
            sx1_ps = misc_ps.tile([BL, 1], f32, tag="misc")
            nc.tensor.matmul(sx1_ps, lhsT=xb_sb, rhs=ones_col, start=True, stop=True)
            sx1 = small.tile([BL, 1], f32, tag="sx1")
            nc.vector.tensor_copy(out=sx1, in_=sx1_ps)

            # softmax over D, then scale by sum(x1)
            neg_max = small.tile([BL, 1], f32, tag="neg_max")
            nc.vector.reduce_max(
                out=neg_max, in_=ot_sb, axis=mybir.AxisListType.X, negate=True
            )
            exp_sb = small.tile([BL, D], f32, tag="exp_sb")
            sum_exp = small.tile([BL, 1], f32, tag="sum_exp")
            nc.scalar.activation(
                out=exp_sb,
                in_=ot_sb,
                func=mybir.ActivationFunctionType.Exp,
                bias=neg_max,
                accum_out=sum_exp,
            )
            rec = small.tile([BL, 1], f32, tag="rec")
            nc.vector.reciprocal(out=rec, in_=sum_exp)
            scale = small.tile([BL, 1], f32, tag="scale")
            nc.vector.tensor_mul(out=scale, in0=rec, in1=sx1)
            out_sb = small.tile([BL, D], f32, tag="out_sb")
            nc.vector.tensor_scalar_mul(out=out_sb, in0=exp_sb, scalar1=scale)
            nc.sync.dma_start(out=out, in_=out_sb)

    nc.finalize()
    return nc


def build_nc_a():
    """Orientation-A bf16 build: s2 tiles in [t, d_out] layout.

    Per (batch, 512-t-block) iteration:
      - SWDGE cast-load x1 tile [128t, 4s, 512d] fp32->bf16
      - x1T chunks via REGULAR bf16 matmuls against identity (N=128 pipelined
        rate; transpose-mode would cost ~350ns/op and skip HAM warmup)
      - two [128, 1024]-bf16 one-bank PSUM tiles -> two DVE copies with
        accum_out (x1 partial sums ride along for free)
      - s2 psum [t=128, 512dout] = sum_dk x1T_chunk.T @ W_h[dk] plus a 5th
        rank-1 accumulation ones.T @ (s1[b]/128 replicated) folding in the
        tanh bias
      - ACT: plain tanh psum -> SBUF bf16
      - V_a contraction on PE: psum_o[1, 512] += va_col.T @ s_tile,
        accumulated over a batch's 16 t-chunks
    Epilogue: softmax on [4, 512] rows + scale by sum(x1).
    """
    import concourse.bass as bass
    import concourse.tile as tile
    from concourse import bacc, mybir
    from concourse.masks import make_identity

    f32 = mybir.dt.float32
    bf16 = mybir.dt.bfloat16

    nc = bacc.Bacc("TRN2", target_bir_lowering=False)

    x0s = nc.dram_tensor("x0s", [BL, D], f32, kind="ExternalInput").ap()
    x1s = nc.dram_tensor("x1s", [BL, T, D], f32, kind="ExternalInput").ap()
    wa = nc.dram_tensor("W_a", [D, D], f32, kind="ExternalInput").ap()
    wh = nc.dram_tensor("W_h", [D, D], f32, kind="ExternalInput").ap()
    va = nc.dram_tensor("V_a", [1, T], f32, kind="ExternalInput").ap()
    out = nc.dram_tensor("out", [BL, D], f32, kind="ExternalOutput").ap()

    with tile.TileContext(nc) as tc:
        with (
            tc.tile_pool(name="consts", bufs=1) as consts,
            tc.tile_pool(name="nat", bufs=3) as nat_pool,
            tc.tile_pool(name="xt", bufs=4) as xt_pool,
            tc.tile_pool(name="s", bufs=4) as s_pool,
            tc.tile_pool(name="small", bufs=1) as small,
            tc.tile_pool(name="dram", bufs=1, space="DRAM") as dram_pool,
            tc.tile_pool(name="misc_ps", bufs=1, space="PSUM") as misc_ps,
            tc.tile_pool(name="tp_ps", bufs=2, space="PSUM") as tp_pool,
            tc.tile_pool(name="mm_ps", bufs=4, space="PSUM") as mm_pool,
            tc.tile_pool(name="o_ps", bufs=1, space="PSUM") as o_pool,
        ):
            # ---------------- constants ----------------
            ident = consts.tile([P, P], bf16, tag="ident")
            make_identity(nc, ident)
            identf = consts.tile([P, P], f32, tag="identf")
            make_identity(nc, identf)
            ones_bf = consts.tile([P, P], bf16, tag="ones_bf")
            nc.vector.memset(ones_bf, 1.0)
            ones_col = consts.tile([P, 1], f32, tag="ones_col")
            nc.vector.memset(ones_col, 1.0)

            wh_sb = consts.tile([P, KC, D], bf16, tag="wh")
            nc.gpsimd.dma_start(out=wh_sb, in_=wh.rearrange("(c p) n -> p c n", p=P))
            wa_sb = consts.tile([P, KC, D], f32, tag="wa")
            nc.sync.dma_start(out=wa_sb, in_=wa.rearrange("(c p) n -> p c n", p=P))

            # va columns: va_sb[p, c] = V_a[c*128 + p]
            va_sb = consts.tile([P, T // P], bf16, tag="va")
            nc.gpsimd.dma_start(
                out=va_sb, in_=va.rearrange("a (c p) -> p (a c)", p=P)
            )
            # va_ind[:, b, c, b'] = va column c if b' == b else 0: batch b's
            # V_a matmuls route their sums into psum row b only
            va_ind = consts.tile([P, BL, T // P, BL], bf16, tag="va_ind")
            nc.vector.memset(va_ind, 0.0)
            for b in range(BL):
                nc.vector.tensor_copy(out=va_ind[:, b, :, b], in_=va_sb)

            # ---------------- phase 0: s1/128 rows + broadcast ----------
            x0_nat = small.tile([P, D], f32, tag="x0_nat")
            nc.vector.memset(x0_nat, 0.0)
            nc.sync.dma_start(out=x0_nat[:BL, :], in_=x0s)
            nc.scalar.mul(out=x0_nat, in_=x0_nat, mul=1.0 / P)
            x0t_sb = small.tile([P, KC, BL], f32, tag="x0t")
            for k in range(KC):
                x0t_ps = misc_ps.tile([P, P], f32, tag="misc")
                nc.tensor.transpose(x0t_ps, x0_nat[:, k * P : (k + 1) * P], identf)
                nc.vector.tensor_copy(out=x0t_sb[:, k, :], in_=x0t_ps[:, :BL])

            s1_ps = misc_ps.tile([BL, D], f32, tag="misc")
            for k in range(KC):
                nc.tensor.matmul(
                    s1_ps,
                    lhsT=x0t_sb[:, k, :],
                    rhs=wa_sb[:, k, :],
                    start=(k == 0),
                    stop=(k == KC - 1),
                )
            s1_row = small.tile([BL, D], f32, tag="s1_row")
            nc.vector.tensor_copy(out=s1_row, in_=s1_ps)
            s1_dram = dram_pool.tile([BL, D], f32, tag="s1_dram")
            nc.sync.dma_start(out=s1_dram, in_=s1_row)
            s1rep = consts.tile([P, BL, D], bf16, tag="s1rep")
            for b in range(BL):
                row = s1_dram[b : b + 1, :]
                bcast = bass.AP(
                    tensor=row.tensor, offset=row.offset, ap=[[0, P], row.ap[-1]]
                )
                nc.gpsimd.dma_start(out=s1rep[:, b, :], in_=bcast)

            # ---------------- main loop ----------------
            xsums = small.tile([P, BL * NTB * KC], f32, tag="xsums")
            ot_sb = small.tile([BL, D], f32, tag="ot_sb")
            o_ps = o_pool.tile([BL, D], f32, tag="o")

            for b in range(BL):
                for tb in range(NTB):
                    nat = nat_pool.tile([P, SUB, D], bf16, tag="nat")
                    src = x1s[b, tb * TBLK : (tb + 1) * TBLK, :].rearrange(
                        "(s p) d -> p s d", p=P
                    )
                    nc.gpsimd.dma_start(out=nat, in_=src)  # SWDGE cast

                    # x1T chunks as regular matmuls vs identity (fp32 PSUM out)
                    xts = []
                    for dk in range(KC):
                        tp_ps = tp_pool.tile([P, TBLK], f32, tag="tp")
                        for s in range(SUB):
                            nc.tensor.matmul(
                                tp_ps[:, s * P : (s + 1) * P],
                                lhsT=nat[:, s, dk * P : (dk + 1) * P],
                                rhs=ident,
                                start=True,
                                stop=True,
                            )
                        xt = xt_pool.tile([P, TBLK], bf16, tag="xt")
                        xi = (b * NTB + tb) * KC + dk
                        nc.vector.tensor_scalar(
                            out=xt,
                            in0=tp_ps,
                            scalar1=0.0,
                            scalar2=0.0,
                            op0=mybir.AluOpType.add,
                            op1=mybir.AluOpType.add,
                            accum_out=xsums[:, xi : xi + 1],
                        )
                        xts.append(xt)

                    # s2 tiles [t=128, dout=512] + rank-1 s1 bias, tanh, V_a
                    for ts_ in range(SUB):
                        mm_ps = mm_pool.tile([P, TBLK], f32, tag="mm")
                        for dk in range(KC):
                            nc.tensor.matmul(
                                mm_ps,
                                lhsT=xts[dk][:, ts_ * P : (ts_ + 1) * P],
                                rhs=wh_sb[:, dk, :],
                                start=(dk == 0),
                                stop=False,
                            )
                        nc.tensor.matmul(
                            mm_ps,
                            lhsT=ones_bf,
                            rhs=s1rep[:, b, :],
                            start=False,
                            stop=True,
                        )
                        s_sb = s_pool.tile([P, TBLK], bf16, tag="s")
                        nc.scalar.activation(
                            out=s_sb,
                            in_=mm_ps,
                            func=mybir.ActivationFunctionType.Tanh,
                        )
                        c = tb * SUB + ts_
                        nc.tensor.matmul(
                            o_ps,
                            lhsT=va_ind[:, b, c, :],
                            rhs=s_sb,
                            start=(b == 0 and c == 0),
                            stop=(b == BL - 1 and c == NTB * SUB - 1),
                            skip_group_check=True,
                        )
            nc.vector.tensor_copy(out=ot_sb, in_=o_ps)

            # ---------------- epilogue ----------------
            xb_sb = small.tile([P, BL], f32, tag="xb_sb")
            nc.vector.reduce_sum(
                out=xb_sb,
                in_=xsums.rearrange("p (b q) -> p b q", q=NTB * KC),
                axis=mybir.AxisListType.X,
            )
            sx1_ps = misc_ps.tile([BL, 1], f32, tag="misc")
            nc.tensor.matmul(sx1_ps, lhsT=xb_sb, rhs=ones_col, start=True, stop=True)
            sx1 = small.tile([BL, 1], f32, tag="sx1")
            nc.vector.tensor_copy(out=sx1, in_=sx1_ps)

            neg_max = small.tile([BL, 1], f32, tag="neg_max")
            nc.vector.reduce_max(
                out=neg_max, in_=ot_sb, axis=mybir.AxisListType.X, negate=True
            )
            exp_sb = small.tile([BL, D], f32, tag="exp_sb")
            sum_exp = small.tile([BL, 1], f32, tag="sum_exp")
            nc.scalar.activation(
                out=exp_sb,
                in_=ot_sb,
                func=mybir.ActivationFunctionType.Exp,
                bias=neg_max,
                accum_out=sum_exp,
            )
            rec = small.tile([BL, 1], f32, tag="rec")
            nc.vector.reciprocal(out=rec, in_=sum_exp)
            scale = small.tile([BL, 1], f32, tag="scale")
            nc.vector.tensor_mul(out=scale, in0=rec, in1=sx1)
            out_sb = small.tile([BL, D], f32, tag="out_sb")
            nc.vector.tensor_scalar_mul(out=out_sb, in0=exp_sb, scalar1=scale)
            nc.sync.dma_start(out=out, in_=out_sb)

    nc.finalize()
    return nc


def build_nc_v2():
    """fp8 DoubleRow main matmul + batched V_a reduction.

    Per (batch, 512-t-block) iteration:
      - SWDGE cast-DMA x1 tile [128t, 4s, 512d] fp32->bf16
      - 16 transpose-matmuls (bf16 vs identity) into ONE psum tile
        tp [128d, 4dk, 512t] (4 banks)
      - copies tp -> xt fp8e4 [128, 4dk, 512]: DVE takes chunks 0-2,
        ACT takes chunk 3; both with accum_out -> bf16-accurate x1 sums
      - main matmul in fp8 DoubleRow: per n-chunk psum [128dout, 512t]
        accumulates 2 MMs, each contracting 256 k (2 k-tiles paired);
        W_h pre-scaled by 64 into fp8 (values ~1.3 rms, safe in e4m3)
      - ACT: tanh(psum * 1/64 + s1T bias) -> s_sb f32 [128, n, tb, 512]
      - after each batch's 4 t-blocks: 4 DVE scalar_tensor_tensor ops of
        FD=2048 (s * va, accum per partition) -> o_parts column
    Epilogue: softmax over D + scale by sum(x1), as baseline.
    """
    import concourse.bass as bass
    import concourse.tile as tile
    from concourse import bacc, mybir
    from concourse.masks import make_identity

    f32 = mybir.dt.float32
    bf16 = mybir.dt.bfloat16
    fp8 = mybir.dt.float8e4
    WSCALE = 64.0

    nc = bacc.Bacc("TRN2", target_bir_lowering=False)

    x0s = nc.dram_tensor("x0s", [BL, D], f32, kind="ExternalInput").ap()
    x1s = nc.dram_tensor("x1s", [BL, T, D], f32, kind="ExternalInput").ap()
    wa = nc.dram_tensor("W_a", [D, D], f32, kind="ExternalInput").ap()
    wh = nc.dram_tensor("W_h", [D, D], f32, kind="ExternalInput").ap()
    va = nc.dram_tensor("V_a", [1, T], f32, kind="ExternalInput").ap()
    out = nc.dram_tensor("out", [BL, D], f32, kind="ExternalOutput").ap()

    with tile.TileContext(nc) as tc:
        with (
            tc.tile_pool(name="consts", bufs=1) as consts,
            tc.tile_pool(name="nat", bufs=3) as nat_pool,
            tc.tile_pool(name="xt", bufs=3) as xt_pool,
            tc.tile_pool(name="s", bufs=2) as s_pool,
            tc.tile_pool(name="scr", bufs=2) as scr_pool,
            tc.tile_pool(name="small", bufs=1) as small,
            tc.tile_pool(name="misc_ps", bufs=1, space="PSUM") as misc_ps,
            tc.tile_pool(name="tp_ps", bufs=1, space="PSUM") as tp_pool,
            tc.tile_pool(name="mm_ps", bufs=3, space="PSUM") as mm_pool,
        ):
            # ---------------- constants ----------------
            ident = consts.tile([P, P], bf16, tag="ident")
            make_identity(nc, ident)
            identf = consts.tile([P, P], f32, tag="identf")
            make_identity(nc, identf)

            # W_h * 64 in fp8e4, k-chunk layout [128, kc, dout]
            wh_f32 = consts.tile([P, KC, D], f32, tag="wh_f32")
            nc.sync.dma_start(out=wh_f32, in_=wh.rearrange("(c p) n -> p c n", p=P))
            wh8 = consts.tile([P, KC, D], fp8, tag="wh8")
            nc.scalar.mul(out=wh8, in_=wh_f32, mul=WSCALE)

            wa_sb = consts.tile([P, KC, D], f32, tag="wa")
            nc.sync.dma_start(out=wa_sb, in_=wa.rearrange("(c p) n -> p c n", p=P))

            va_sb = consts.tile([P, NTB, TBLK], f32, tag="va")
            va_bcast = bass.AP(
                tensor=va.tensor, offset=va.offset, ap=[[0, P], va.ap[-1]]
            )
            nc.gpsimd.dma_start(
                out=va_sb.rearrange("p a b -> p (a b)"), in_=va_bcast
            )

            ones_col = consts.tile([P, 1], f32, tag="ones_col")
            nc.vector.memset(ones_col, 1.0)

            # ---------------- phase 0: s1T = (x0 @ W_a)^T ----------------
            x0_nat = small.tile([P, D], f32, tag="x0_nat")
            nc.vector.memset(x0_nat, 0.0)
            nc.sync.dma_start(out=x0_nat[:BL, :], in_=x0s)
            x0t_sb = small.tile([P, KC, BL], f32, tag="x0t")
            for k in range(KC):
                x0t_ps = misc_ps.tile([P, P], f32, tag="misc")
                nc.tensor.transpose(
                    x0t_ps, x0_nat[:, k * P : (k + 1) * P], identf
                )
                nc.vector.tensor_copy(out=x0t_sb[:, k, :], in_=x0t_ps[:, :BL])

            s1t_sb = small.tile([P, NCH, BL], f32, tag="s1t")
            for n in range(NCH):
                s1_ps = misc_ps.tile([P, BL], f32, tag="misc")
                for k in range(KC):
                    nc.tensor.matmul(
                        s1_ps,
                        lhsT=wa_sb[:, k, n * P : (n + 1) * P],
                        rhs=x0t_sb[:, k, :],
                        start=(k == 0),
                        stop=(k == KC - 1),
                    )
                nc.vector.tensor_copy(out=s1t_sb[:, n, :], in_=s1_ps)

            # ---------------- main loop ----------------
            o_parts = small.tile([P, NCH * BL], f32, tag="o_parts")
            xsums = small.tile([P, 2 * BL * NTB], f32, tag="xsums")

            for b in range(BL):
                s_sb = s_pool.tile([P, NCH, NTB, TBLK], f32, tag="s")
                for tb in range(NTB):
                    it = b * NTB + tb
                    nat = nat_pool.tile([P, SUB, D], bf16, tag="nat")
                    src = x1s[b, tb * TBLK : (tb + 1) * TBLK, :].rearrange(
                        "(s p) d -> p s d", p=P
                    )
                    nc.gpsimd.dma_start(out=nat, in_=src)  # SWDGE cast

                    # x1 tile -> transposed chunks in one 4-bank psum tile
                    tp = tp_pool.tile([P, KC, TBLK], f32, tag="tp")
                    for dk in range(KC):
                        for s in range(SUB):
                            nc.tensor.matmul(
                                tp[:, dk, s * P : (s + 1) * P],
                                lhsT=nat[:, s, dk * P : (dk + 1) * P],
                                rhs=ident,
                                start=True,
                                stop=True,
                            )

                    # psum -> SBUF fp8 casts (+ x1 partial sums via accum)
                    xt = xt_pool.tile([P, KC, TBLK], fp8, tag="xt")
                    nc.vector.tensor_scalar(
                        out=xt[:, 0:3, :],
                        in0=tp[:, 0:3, :],
                        scalar1=0.0,
                        scalar2=0.0,
                        op0=mybir.AluOpType.add,
                        op1=mybir.AluOpType.add,
                        accum_out=xsums[:, 2 * it : 2 * it + 1],
                    )
                    nc.scalar.activation(
                        out=xt[:, 3, :],
                        in_=tp[:, 3, :],
                        func=mybir.ActivationFunctionType.Copy,
                        accum_out=xsums[:, 2 * it + 1 : 2 * it + 2],
                    )

                    # main matmul: fp8 DoubleRow, 2 MMs of K=256 per n-chunk
                    for n in range(NCH):
                        mm = mm_pool.tile([P, TBLK], f32, tag="mm")
                        for q in range(2):
                            nc.tensor.matmul(
                                mm,
                                lhsT=wh8[:, 2 * q : 2 * q + 2, n * P : (n + 1) * P],
                                rhs=xt[:, 2 * q : 2 * q + 2, :],
                                start=(q == 0),
                                stop=(q == 1),
                                perf_mode=mybir.MatmulPerfMode.DoubleRow,
                            )
                        nc.scalar.activation(
                            out=s_sb[:, n, tb, :],
                            in_=mm,
                            func=mybir.ActivationFunctionType.Tanh,
                            bias=s1t_sb[:, n, b : b + 1],
                            scale=1.0 / WSCALE,
                        )

                # V_a-weighted t-sums over the whole batch (FD=2048)
                for n in range(NCH):
                    scr = scr_pool.tile([P, NTB, TBLK], f32, tag="scr")
                    idx = n * BL + b
                    nc.vector.scalar_tensor_tensor(
                        out=scr,
                        in0=s_sb[:, n, :, :],
                        scalar=1.0,
                        in1=va_sb,
                        op0=mybir.AluOpType.mult,
                        op1=mybir.AluOpType.mult,
                        accum_out=o_parts[:, idx : idx + 1],
                    )

            # ---------------- epilogue ----------------
            # transpose o -> [b, d]
            o3 = o_parts.rearrange("p (n b) -> p n b", b=BL)
            ot_ps = misc_ps.tile([BL, D], f32, tag="misc")
            for n in range(NCH):
                nc.tensor.transpose(
                    ot_ps[:, n * P : (n + 1) * P], o3[:, n, :], identf
                )
            ot_sb = small.tile([BL, D], f32, tag="ot_sb")
            nc.vector.tensor_copy(out=ot_sb, in_=ot_ps)

            # sum(x1[b]): reduce per-batch accum columns, then contract d
            xb_sb = small.tile([P, BL], f32, tag="xb_sb")
            nc.vector.reduce_sum(
                out=xb_sb,
                in_=xsums.rearrange("p (b q) -> p b q", q=2 * NTB),
                axis=mybir.AxisListType.X,
            )
            sx1_ps = misc_ps.tile([BL, 1], f32, tag="misc")
            nc.tensor.matmul(sx1_ps, lhsT=xb_sb, rhs=ones_col, start=True, stop=True)
            sx1 = small.tile([BL, 1], f32, tag="sx1")
            nc.vector.tensor_copy(out=sx1, in_=sx1_ps)

            # softmax over D, then scale by sum(x1)
            neg_max = small.tile([BL, 1], f32, tag="neg_max")
            nc.vector.reduce_max(
                out=neg_max, in_=ot_sb, axis=mybir.AxisListType.X, negate=True
            )
            exp_sb = small.tile([BL, D], f32, tag="exp_sb")
            sum_exp = small.tile([BL, 1], f32, tag="sum_exp")
            nc.scalar.activation(
                out=exp_sb,
                in_=ot_sb,
                func=mybir.ActivationFunctionType.Exp,
                bias=neg_max,
                accum_out=sum_exp,
            )
            rec = small.tile([BL, 1], f32, tag="rec")
            nc.vector.reciprocal(out=rec, in_=sum_exp)
            scale = small.tile([BL, 1], f32, tag="scale")
            nc.vector.tensor_mul(out=scale, in0=rec, in1=sx1)
            out_sb = small.tile([BL, D], f32, tag="out_sb")
            nc.vector.tensor_scalar_mul(out=out_sb, in0=exp_sb, scalar1=scale)
            nc.sync.dma_start(out=out, in_=out_sb)

    nc.finalize()
    return nc


def make_in_maps(x0, x1, W_a, W_h, V_a):
    x0 = np.ascontiguousarray(x0, dtype=np.float32)
    x1 = np.ascontiguousarray(x1, dtype=np.float32)
    W_a = np.ascontiguousarray(W_a, dtype=np.float32)
    W_h = np.ascontiguousarray(W_h, dtype=np.float32)
    V_a = np.ascontiguousarray(V_a, dtype=np.float32)
    in_maps = []
    for c in range(NCORES):
        sl = slice(c * BL, (c + 1) * BL)
        in_maps.append(
            {
                "x0s": np.ascontiguousarray(x0[sl]),
                "x1s": np.ascontiguousarray(x1[sl]),
                "W_a": W_a,
                "W_h": W_h,
                "V_a": V_a,
            }
        )
    return in_maps


_NC_CACHE = {}


def _build(mode):
    if mode == "v2":
        return build_nc_v2()
    if mode == "a":
        return build_nc_a()
    return build_nc(mode)


def kernel(x0, x1, W_a, W_h, V_a):
    from concourse.bass_utils import run_bass_kernel_spmd

    mode = "v2"
    nc = _NC_CACHE.get(mode)
    if nc is None:
        nc = _NC_CACHE[mode] = _build(mode)
    in_maps = make_in_maps(x0, x1, W_a, W_h, V_a)
    res = run_bass_kernel_spmd(nc, in_maps, core_ids=list(range(NCORES)))
    return np.concatenate([res.results[c]["out"] for c in range(NCORES)], axis=0)



# revision 5
# speedup vs baseline: 1.1673x; 1.1673x over previous
"""Trainium2 Bass kernel for nn_AttentionLayer (B=32, T=2048, D=512).

Computation (per batch b):
    s1 = x0 @ W_a                       # (D,)
    s2 = x1[b] @ W_h                    # (T, D)
    s  = tanh(s1 + s2)                  # (T, D)
    o  = V_a @ s                        # (D,)   (contract T)
    alpha = softmax(o)                  # over D
    out[b] = alpha * sum(x1[b])

Sharding: data-parallel over batch across 8 NeuronCores (4 batches/core),
weights replicated, no cross-core communication.

Per-core dataflow, default mode "bf16" (measured 128 us on TRN2, rel err
~2e-3; mode "f32" is the full-precision fallback at ~150 us, rel err ~5e-5):
  - SWDGE cast-DMA x1 natural tile [128t, 4s, 512d] fp32->bf16 into SBUF
  - x1 -> x1T chunks [128d, 512t] via REGULAR bf16 matmuls against an
    identity (1 cyc/row, keeps the PE HAM warm; transpose-mode measures
    ~350 ns/op and regular fp32 matmuls are 4 cyc/row)
  - PSUM->SBUF copies of x1T split across ACT/DVE; their accum_out emits
    per-partition partial sums of x1 for free (-> sum(x1[b]) later)
  - PE matmul (bf16): s2T[dout, t] = sum_k W_h[k, dout] * x1T[k, t]
  - ACT: tanh(s2T + bias s1T[dout, 1]) fused, PSUM -> SBUF
  - DVE scalar_tensor_tensor with partition-broadcast V_a: weighted t-sums
    (InstTensorTensorReduce and all-bf16 ScalarTensorTensor+accum both
    wedge TRN2; fp32 ScalarTensorTensor is the working form)
  - Epilogue: softmax over D on [4, 512] rows, scale by sum(x1[b])
"""

import numpy as np

B, T, D = 32, 2048, 512
NCORES = 8
BL = B // NCORES          # batches per core
P = 128
KC = D // P               # k chunks (4)
NCH = D // P              # output-d chunks (4)
TBLK = 512                # t elements per block
NTB = T // TBLK           # t blocks (4)
SUB = TBLK // P           # natural subtiles per block (4)


def build_nc(mode: str = "f32"):
    """Build the per-core Bass program. mode in {"f32", "bf16"}."""
    import concourse.bass as bass
    import concourse.tile as tile
    from concourse import bacc, mybir
    from concourse.masks import make_identity

    f32 = mybir.dt.float32
    f32r = mybir.dt.float32r
    bf16 = mybir.dt.bfloat16
    dt_nat = f32 if mode == "f32" else bf16     # natural x1 tiles
    dt_mm = f32r if mode == "f32" else bf16     # matmul operand tiles
    # tanh output / va / scr stay fp32: the bf16 ScalarTensorTensor+accum
    # path wedges TRN2 hardware (perf-mode + accumulator readout)
    dt_s = f32

    nc = bacc.Bacc("TRN2", target_bir_lowering=False)

    x0s = nc.dram_tensor("x0s", [BL, D], f32, kind="ExternalInput").ap()
    x1s = nc.dram_tensor("x1s", [BL, T, D], f32, kind="ExternalInput").ap()
    wa = nc.dram_tensor("W_a", [D, D], f32, kind="ExternalInput").ap()
    wh = nc.dram_tensor("W_h", [D, D], f32, kind="ExternalInput").ap()
    va = nc.dram_tensor("V_a", [1, T], f32, kind="ExternalInput").ap()
    out = nc.dram_tensor("out", [BL, D], f32, kind="ExternalOutput").ap()

    with tile.TileContext(nc) as tc:
        with (
            tc.tile_pool(name="consts", bufs=1) as consts,
            tc.tile_pool(name="nat", bufs=3) as nat_pool,
            tc.tile_pool(name="xt", bufs=8) as xt_pool,
            tc.tile_pool(name="s", bufs=4) as s_pool,
            tc.tile_pool(name="scr", bufs=2) as scr_pool,
            tc.tile_pool(name="small", bufs=1) as small,
            tc.tile_pool(name="misc_ps", bufs=1, space="PSUM") as misc_ps,
            tc.tile_pool(name="tp_ps", bufs=2, space="PSUM") as tp_pool,
            tc.tile_pool(name="mm_ps", bufs=4, space="PSUM") as mm_pool,
        ):
            # ---------------- constants ----------------
            ident = consts.tile([P, P], dt_nat, tag="ident")
            make_identity(nc, ident)
            identf = ident
            if dt_nat != f32:
                identf = consts.tile([P, P], f32, tag="identf")
                make_identity(nc, identf)

            # W_h in matmul dtype (f32r needs an explicit rounding producer)
            wh_sb = consts.tile([P, KC, D], dt_mm, tag="wh")
            if mode == "f32":
                wh_f32 = consts.tile([P, KC, D], f32, tag="wh_f32")
                nc.sync.dma_start(
                    out=wh_f32, in_=wh.rearrange("(c p) n -> p c n", p=P)
                )
                nc.vector.tensor_copy(out=wh_sb, in_=wh_f32)
            else:
                nc.gpsimd.dma_start(
                    out=wh_sb, in_=wh.rearrange("(c p) n -> p c n", p=P)
                )
            wa_sb = consts.tile([P, KC, D], f32, tag="wa")
            nc.sync.dma_start(out=wa_sb, in_=wa.rearrange("(c p) n -> p c n", p=P))

            va_sb = consts.tile([P, T], dt_s, tag="va")
            va_bcast = bass.AP(
                tensor=va.tensor, offset=va.offset, ap=[[0, P], va.ap[-1]]
            )
            nc.gpsimd.dma_start(out=va_sb, in_=va_bcast)

            # all-ones column for the final cross-partition sum
            ones_col = consts.tile([P, 1], f32, tag="ones_col")
            nc.vector.memset(ones_col, 1.0)

            # ---------------- phase 0: s1T = (x0 @ W_a)^T ----------------
            # pad x0 to 128 partitions: contraction dims < 128 are flaky on HW
            x0_nat = small.tile([P, D], f32, tag="x0_nat")
            nc.vector.memset(x0_nat, 0.0)
            nc.sync.dma_start(out=x0_nat[:BL, :], in_=x0s)
            x0t_sb = small.tile([P, KC, BL], f32, tag="x0t")
            for k in range(KC):
                x0t_ps = misc_ps.tile([P, P], f32, tag="misc")
                nc.tensor.transpose(
                    x0t_ps, x0_nat[:, k * P : (k + 1) * P], identf
                )
                nc.vector.tensor_copy(out=x0t_sb[:, k, :], in_=x0t_ps[:, :BL])

            s1t_sb = small.tile([P, NCH, BL], f32, tag="s1t")
            for n in range(NCH):
                s1_ps = misc_ps.tile([P, BL], f32, tag="misc")
                for k in range(KC):
                    nc.tensor.matmul(
                        s1_ps,
                        lhsT=wa_sb[:, k, n * P : (n + 1) * P],
                        rhs=x0t_sb[:, k, :],
                        start=(k == 0),
                        stop=(k == KC - 1),
                    )
                nc.vector.tensor_copy(out=s1t_sb[:, n, :], in_=s1_ps)

            # ---------------- main loop ----------------
            o_parts = small.tile([P, NCH * BL * NTB], f32, tag="o_parts")
            # per-(b,tb,dk) partial sums of x1 (per d-partition), free via the
            # accum_out of the transpose-copy ops
            xsums = small.tile([P, BL * NTB * KC], f32, tag="xsums")

            for b in range(BL):
                for tb in range(NTB):
                    nat = nat_pool.tile([P, SUB, D], dt_nat, tag="nat")
                    src = x1s[b, tb * TBLK : (tb + 1) * TBLK, :].rearrange(
                        "(s p) d -> p s d", p=P
                    )
                    if dt_nat == f32:
                        nc.sync.dma_start(out=nat, in_=src)
                    else:
                        nc.gpsimd.dma_start(out=nat, in_=src)  # SWDGE cast

                    # transpose x1 tile: [t, d] -> 4 chunks [128d, 512t];
                    # the PSUM->SBUF copies also emit per-partition sums.
                    # bf16: regular matmuls vs identity (1 cyc/row, keeps HAM
                    # warm) into fp32 PSUM; f32: transpose-mode.
                    xts = []
                    for dk in range(KC):
                        if dt_nat == bf16:
                            tp_ps = tp_pool.tile([P, TBLK], f32, tag="tp")
                            for s in range(SUB):
                                nc.tensor.matmul(
                                    tp_ps[:, s * P : (s + 1) * P],
                                    lhsT=nat[:, s, dk * P : (dk + 1) * P],
                                    rhs=ident,
                                    start=True,
                                    stop=True,
                                )
                        else:
                            tp_ps = tp_pool.tile([P, TBLK], dt_nat, tag="tp")
                            for s in range(SUB):
                                nc.tensor.transpose(
                                    tp_ps[:, s * P : (s + 1) * P],
                                    nat[:, s, dk * P : (dk + 1) * P],
                                    ident,
                                )
                        xt = xt_pool.tile([P, TBLK], dt_mm, tag="xt")
                        xi = (b * NTB + tb) * KC + dk
                        if dk < 2:
                            nc.scalar.activation(
                                out=xt,
                                in_=tp_ps,
                                func=mybir.ActivationFunctionType.Copy,
                                accum_out=xsums[:, xi : xi + 1],
                            )
                        else:
                            nc.vector.tensor_scalar(
                                out=xt,
                                in0=tp_ps,
                                scalar1=0.0,
                                scalar2=0.0,
                                op0=mybir.AluOpType.add,
                                op1=mybir.AluOpType.add,
                                accum_out=xsums[:, xi : xi + 1],
                            )
                        xts.append(xt)

                    # s2T = W_h^T x1^T ; tanh(+s1T bias) ; V_a-weighted t-sum
                    for n in range(NCH):
                        mm_ps = mm_pool.tile([P, TBLK], f32, tag="mm")
                        for dk in range(KC):
                            nc.tensor.matmul(
                                mm_ps,
                                lhsT=wh_sb[:, dk, n * P : (n + 1) * P],
                                rhs=xts[dk][:],
                                start=(dk == 0),
                                stop=(dk == KC - 1),
                            )
                        s_sb = s_pool.tile([P, TBLK], dt_s, tag="s")
                        nc.scalar.activation(
                            out=s_sb,
                            in_=mm_ps,
                            func=mybir.ActivationFunctionType.Tanh,
                            bias=s1t_sb[:, n, b : b + 1],
                        )
                        scr = scr_pool.tile([P, TBLK], dt_s, tag="scr")
                        idx = (n * BL + b) * NTB + tb
                        # out = (s * 1.0) * va ; accum_out = sum(out) per
                        # partition. (InstTensorTensorReduce wedges TRN2 here;
                        # InstScalarTensorTensor is the working equivalent.)
                        nc.vector.scalar_tensor_tensor(
                            out=scr,
                            in0=s_sb,
                            scalar=1.0,
                            in1=va_sb[:, tb * TBLK : (tb + 1) * TBLK],
                            op0=mybir.AluOpType.mult,
                            op1=mybir.AluOpType.mult,
                            accum_out=o_parts[:, idx : idx + 1],
                        )

            # ---------------- epilogue ----------------
            # o (transposed layout) = sum of partials over t-blocks
            o_sb = small.tile([P, NCH * BL], f32, tag="o_sb")
            nc.vector.reduce_sum(
                out=o_sb,
                in_=o_parts.rearrange("p (q t) -> p q t", t=NTB),
                axis=mybir.AxisListType.X,
            )
            # transpose o -> [b, d]
            o3 = o_sb.rearrange("p (n b) -> p n b", b=BL)
            ot_ps = misc_ps.tile([BL, D], f32, tag="misc")
            for n in range(NCH):
                nc.tensor.transpose(
                    ot_ps[:, n * P : (n + 1) * P], o3[:, n, :], identf
                )
            ot_sb = small.tile([BL, D], f32, tag="ot_sb")
            nc.vector.tensor_copy(out=ot_sb, in_=ot_ps)

            # sum(x1[b]) finish: reduce per-batch columns, then contract the
            # partition (d) axis with a ones-vector matmul -> [BL, 1]
            xb_sb = small.tile([P, BL], f32, tag="xb_sb")
            nc.vector.reduce_sum(
                out=xb_sb,
                in_=xsums.rearrange("p (b q) -> p b q", q=NTB * KC),
                axis=mybir.AxisListType.X,
            )
            sx1_ps = misc_ps.tile([BL, 1], f32, tag="misc")
            nc.tensor.matmul(sx1_ps, lhsT=xb_sb, rhs=ones_col, start=True, stop=True)
            sx1 = small.tile([BL, 1], f32, tag="sx1")
            nc.vector.tensor_copy(out=sx1, in_=sx1_ps)

            # softmax over D, then scale by sum(x1)
            neg_max = small.tile([BL, 1], f32, tag="neg_max")
            nc.vector.reduce_max(
                out=neg_max, in_=ot_sb, axis=mybir.AxisListType.X, negate=True
            )
            exp_sb = small.tile([BL, D], f32, tag="exp_sb")
            sum_exp = small.tile([BL, 1], f32, tag="sum_exp")
            nc.scalar.activation(
                out=exp_sb,
                in_=ot_sb,
                func=mybir.ActivationFunctionType.Exp,
                bias=neg_max,
                accum_out=sum_exp,
            )
            rec = small.tile([BL, 1], f32, tag="rec")
            nc.vector.reciprocal(out=rec, in_=sum_exp)
            scale = small.tile([BL, 1], f32, tag="scale")
            nc.vector.tensor_mul(out=scale, in0=rec, in1=sx1)
            out_sb = small.tile([BL, D], f32, tag="out_sb")
            nc.vector.tensor_scalar_mul(out=out_sb, in0=exp_sb, scalar1=scale)
            nc.sync.dma_start(out=out, in_=out_sb)

    nc.finalize()
    return nc


def build_nc_a():
    """Orientation-A bf16 build: s2 tiles in [t, d_out] layout.

    Per (batch, 512-t-block) iteration:
      - SWDGE cast-load x1 tile [128t, 4s, 512d] fp32->bf16
      - x1T chunks via REGULAR bf16 matmuls against identity (N=128 pipelined
        rate; transpose-mode would cost ~350ns/op and skip HAM warmup)
      - two [128, 1024]-bf16 one-bank PSUM tiles -> two DVE copies with
        accum_out (x1 partial sums ride along for free)
      - s2 psum [t=128, 512dout] = sum_dk x1T_chunk.T @ W_h[dk] plus a 5th
        rank-1 accumulation ones.T @ (s1[b]/128 replicated) folding in the
        tanh bias
      - ACT: plain tanh psum -> SBUF bf16
      - V_a contraction on PE: psum_o[1, 512] += va_col.T @ s_tile,
        accumulated over a batch's 16 t-chunks
    Epilogue: softmax on [4, 512] rows + scale by sum(x1).
    """
    import concourse.bass as bass
    import concourse.tile as tile
    from concourse import bacc, mybir
    from concourse.masks import make_identity

    f32 = mybir.dt.float32
    bf16 = mybir.dt.bfloat16

    nc = bacc.Bacc("TRN2", target_bir_lowering=False)

    x0s = nc.dram_tensor("x0s", [BL, D], f32, kind="ExternalInput").ap()
    x1s = nc.dram_tensor("x1s", [BL, T, D], f32, kind="ExternalInput").ap()
    wa = nc.dram_tensor("W_a", [D, D], f32, kind="ExternalInput").ap()
    wh = nc.dram_tensor("W_h", [D, D], f32, kind="ExternalInput").ap()
    va = nc.dram_tensor("V_a", [1, T], f32, kind="ExternalInput").ap()
    out = nc.dram_tensor("out", [BL, D], f32, kind="ExternalOutput").ap()

    with tile.TileContext(nc) as tc:
        with (
            tc.tile_pool(name="consts", bufs=1) as consts,
            tc.tile_pool(name="nat", bufs=3) as nat_pool,
            tc.tile_pool(name="xt", bufs=4) as xt_pool,
            tc.tile_pool(name="s", bufs=4) as s_pool,
            tc.tile_pool(name="small", bufs=1) as small,
            tc.tile_pool(name="dram", bufs=1, space="DRAM") as dram_pool,
            tc.tile_pool(name="misc_ps", bufs=1, space="PSUM") as misc_ps,
            tc.tile_pool(name="tp_ps", bufs=2, space="PSUM") as tp_pool,
            tc.tile_pool(name="mm_ps", bufs=4, space="PSUM") as mm_pool,
            tc.tile_pool(name="o_ps", bufs=1, space="PSUM") as o_pool,
        ):
            # ---------------- constants ----------------
            ident = consts.tile([P, P], bf16, tag="ident")
            make_identity(nc, ident)
            identf = consts.tile([P, P], f32, tag="identf")
            make_identity(nc, identf)
            ones_bf = consts.tile([P, P], bf16, tag="ones_bf")
            nc.vector.memset(ones_bf, 1.0)
            ones_col = consts.tile([P, 1], f32, tag="ones_col")
            nc.vector.memset(ones_col, 1.0)

            wh_sb = consts.tile([P, KC, D], bf16, tag="wh")
            nc.gpsimd.dma_start(out=wh_sb, in_=wh.rearrange("(c p) n -> p c n", p=P))
            wa_sb = consts.tile([P, KC, D], f32, tag="wa")
            nc.sync.dma_start(out=wa_sb, in_=wa.rearrange("(c p) n -> p c n", p=P))

            # va columns: va_sb[p, c] = V_a[c*128 + p]
            va_sb = consts.tile([P, T // P], bf16, tag="va")
            nc.gpsimd.dma_start(
                out=va_sb, in_=va.rearrange("a (c p) -> p (a c)", p=P)
            )
            # va_ind[:, b, c, b'] = va column c if b' == b else 0: batch b's
            # V_a matmuls route their sums into psum row b only
            va_ind = consts.tile([P, BL, T // P, BL], bf16, tag="va_ind")
            nc.vector.memset(va_ind, 0.0)
            for b in range(BL):
                nc.vector.tensor_copy(out=va_ind[:, b, :, b], in_=va_sb)

            # ---------------- phase 0: s1/128 rows + broadcast ----------
            x0_nat = small.tile([P, D], f32, tag="x0_nat")
            nc.vector.memset(x0_nat, 0.0)
            nc.sync.dma_start(out=x0_nat[:BL, :], in_=x0s)
            nc.scalar.mul(out=x0_nat, in_=x0_nat, mul=1.0 / P)
            x0t_sb = small.tile([P, KC, BL], f32, tag="x0t")
            for k in range(KC):
                x0t_ps = misc_ps.tile([P, P], f32, tag="misc")
                nc.tensor.transpose(x0t_ps, x0_nat[:, k * P : (k + 1) * P], identf)
                nc.vector.tensor_copy(out=x0t_sb[:, k, :], in_=x0t_ps[:, :BL])

            s1_ps = misc_ps.tile([BL, D], f32, tag="misc")
            for k in range(KC):
                nc.tensor.matmul(
                    s1_ps,
                    lhsT=x0t_sb[:, k, :],
                    rhs=wa_sb[:, k, :],
                    start=(k == 0),
                    stop=(k == KC - 1),
                )
            s1_row = small.tile([BL, D], f32, tag="s1_row")
            nc.vector.tensor_copy(out=s1_row, in_=s1_ps)
            s1_dram = dram_pool.tile([BL, D], f32, tag="s1_dram")
            nc.sync.dma_start(out=s1_dram, in_=s1_row)
            s1rep = consts.tile([P, BL, D], bf16, tag="s1rep")
            for b in range(BL):
                row = s1_dram[b : b + 1, :]
                bcast = bass.AP(
                    tensor=row.tensor, offset=row.offset, ap=[[0, P], row.ap[-1]]
                )
                nc.gpsimd.dma_start(out=s1rep[:, b, :], in_=bcast)

            # ---------------- main loop ----------------
            xsums = small.tile([P, BL * NTB * KC], f32, tag="xsums")
            ot_sb = small.tile([BL, D], f32, tag="ot_sb")
            o_ps = o_pool.tile([BL, D], f32, tag="o")

            for b in range(BL):
                for tb in range(NTB):
                    nat = nat_pool.tile([P, SUB, D], bf16, tag="nat")
                    src = x1s[b, tb * TBLK : (tb + 1) * TBLK, :].rearrange(
                        "(s p) d -> p s d", p=P
                    )
                    nc.gpsimd.dma_start(out=nat, in_=src)  # SWDGE cast

                    # x1T chunks as regular matmuls vs identity (fp32 PSUM out)
                    xts = []
                    for dk in range(KC):
                        tp_ps = tp_pool.tile([P, TBLK], f32, tag="tp")
                        for s in range(SUB):
                            nc.tensor.matmul(
                                tp_ps[:, s * P : (s + 1) * P],
                                lhsT=nat[:, s, dk * P : (dk + 1) * P],
                                rhs=ident,
                                start=True,
                                stop=True,
                            )
                        xt = xt_pool.tile([P, TBLK], bf16, tag="xt")
                        xi = (b * NTB + tb) * KC + dk
                        nc.vector.tensor_scalar(
                            out=xt,
                            in0=tp_ps,
                            scalar1=0.0,
                            scalar2=0.0,
                            op0=mybir.AluOpType.add,
                            op1=mybir.AluOpType.add,
                            accum_out=xsums[:, xi : xi + 1],
                        )
                        xts.append(xt)

                    # s2 tiles [t=128, dout=512] + rank-1 s1 bias, tanh, V_a
                    for ts_ in range(SUB):
                        mm_ps = mm_pool.tile([P, TBLK], f32, tag="mm")
                        for dk in range(KC):
                            nc.tensor.matmul(
                                mm_ps,
                                lhsT=xts[dk][:, ts_ * P : (ts_ + 1) * P],
                                rhs=wh_sb[:, dk, :],
                                start=(dk == 0),
                                stop=False,
                            )
                        nc.tensor.matmul(
                            mm_ps,
                            lhsT=ones_bf,
                            rhs=s1rep[:, b, :],
                            start=False,
                            stop=True,
                        )
                        s_sb = s_pool.tile([P, TBLK], bf16, tag="s")
                        nc.scalar.activation(
                            out=s_sb,
                            in_=mm_ps,
                            func=mybir.ActivationFunctionType.Tanh,
                        )
                        c = tb * SUB + ts_
                        nc.tensor.matmul(
                            o_ps,
                            lhsT=va_ind[:, b, c, :],
                            rhs=s_sb,
                            start=(b == 0 and c == 0),
                            stop=(b == BL - 1 and c == NTB * SUB - 1),
                            skip_group_check=True,
                        )
            nc.vector.tensor_copy(out=ot_sb, in_=o_ps)

            # ---------------- epilogue ----------------
            xb_sb = small.tile([P, BL], f32, tag="xb_sb")
            nc.vector.reduce_sum(
                out=xb_sb,
                in_=xsums.rearrange("p (b q) -> p b q", q=NTB * KC),
                axis=mybir.AxisListType.X,
            )
            sx1_ps = misc_ps.tile([BL, 1], f32, tag="misc")
            nc.tensor.matmul(sx1_ps, lhsT=xb_sb, rhs=ones_col, start=True, stop=True)
            sx1 = small.tile([BL, 1], f32, tag="sx1")
            nc.vector.tensor_copy(out=sx1, in_=sx1_ps)

            neg_max = small.tile([BL, 1], f32, tag="neg_max")
            nc.vector.reduce_max(
                out=neg_max, in_=ot_sb, axis=mybir.AxisListType.X, negate=True
            )
            exp_sb = small.tile([BL, D], f32, tag="exp_sb")
            sum_exp = small.tile([BL, 1], f32, tag="sum_exp")
            nc.scalar.activation(
                out=exp_sb,
                in_=ot_sb,
                func=mybir.ActivationFunctionType.Exp,
                bias=neg_max,
                accum_out=sum_exp,
            )
            rec = small.tile([BL, 1], f32, tag="rec")
            nc.vector.reciprocal(out=rec, in_=sum_exp)
            scale = small.tile([BL, 1], f32, tag="scale")
            nc.vector.tensor_mul(out=scale, in0=rec, in1=sx1)
            out_sb = small.tile([BL, D], f32, tag="out_sb")
            nc.vector.tensor_scalar_mul(out=out_sb, in0=exp_sb, scalar1=scale)
            nc.sync.dma_start(out=out, in_=out_sb)

    nc.finalize()
    return nc


def build_nc_v2():
    """fp8 DoubleRow main matmul + batched V_a reduction.

    Per (batch, 512-t-block) iteration:
      - SWDGE cast-DMA x1 tile [128t, 4s, 512d] fp32->bf16
      - 16 transpose-matmuls (bf16 vs identity) into ONE psum tile
        tp [128d, 4dk, 512t] (4 banks)
      - copies tp -> xt fp8e4 [128, 4dk, 512]: DVE takes chunks 0-2,
        ACT takes chunk 3; both with accum_out -> bf16-accurate x1 sums
      - main matmul in fp8 DoubleRow: per n-chunk psum [128dout, 512t]
        accumulates 2 MMs, each contracting 256 k (2 k-tiles paired);
        W_h pre-scaled by 64 into fp8 (values ~1.3 rms, safe in e4m3)
      - ACT: tanh(psum * 1/64 + s1T bias) -> s_sb f32 [128, n, tb, 512]
      - after each batch's 4 t-blocks: 4 DVE scalar_tensor_tensor ops of
        FD=2048 (s * va, accum per partition) -> o_parts column
    Epilogue: softmax over D + scale by sum(x1), as baseline.
    """
    import concourse.bass as bass
    import concourse.tile as tile
    from concourse import bacc, mybir
    from concourse.masks import make_identity

    f32 = mybir.dt.float32
    bf16 = mybir.dt.bfloat16
    fp8 = mybir.dt.float8e4
    WSCALE = 64.0

    nc = bacc.Bacc("TRN2", target_bir_lowering=False)

    x0s = nc.dram_tensor("x0s", [BL, D], f32, kind="ExternalInput").ap()
    x1s = nc.dram_tensor("x1s", [BL, T, D], f32, kind="ExternalInput").ap()
    wa = nc.dram_tensor("W_a", [D, D], f32, kind="ExternalInput").ap()
    wh = nc.dram_tensor("W_h", [D, D], f32, kind="ExternalInput").ap()
    va = nc.dram_tensor("V_a", [1, T], f32, kind="ExternalInput").ap()
    out = nc.dram_tensor("out", [BL, D], f32, kind="ExternalOutput").ap()

    with tile.TileContext(nc) as tc:
        with (
            tc.tile_pool(name="consts", bufs=1) as consts,
            tc.tile_pool(name="nat", bufs=4) as nat_pool,
            tc.tile_pool(name="xt", bufs=3) as xt_pool,
            tc.tile_pool(name="s", bufs=2) as s_pool,
            tc.tile_pool(name="scr", bufs=2) as scr_pool,
            tc.tile_pool(name="small", bufs=1) as small,
            tc.tile_pool(name="misc_ps", bufs=1, space="PSUM") as misc_ps,
            tc.tile_pool(name="tpa_ps", bufs=1, space="PSUM") as tpa_pool,
            tc.tile_pool(name="tpb_ps", bufs=1, space="PSUM") as tpb_pool,
            tc.tile_pool(name="mm_ps", bufs=3, space="PSUM") as mm_pool,
        ):
            # ---------------- prefetch x1 tiles, then constants ----------
            # Issue the first nat DMAs before anything else lands on the
            # SWDGE queue so the PE can start transposing ASAP.
            NPRE = 3
            prenat = []
            for i in range(NPRE):
                b, tb = divmod(i, NTB)
                nat = nat_pool.tile([P, SUB, D], bf16, tag="nat")
                src = x1s[b, tb * TBLK : (tb + 1) * TBLK, :].rearrange(
                    "(s p) d -> p s d", p=P
                )
                nc.gpsimd.dma_start(out=nat, in_=src)
                prenat.append(nat)

            ident = consts.tile([P, P], bf16, tag="ident")
            make_identity(nc, ident)
            identf = consts.tile([P, P], f32, tag="identf")
            make_identity(nc, identf)

            # W_h * 64 in fp8e4, k-chunk layout [128, kc, dout]
            wh_f32 = consts.tile([P, KC, D], f32, tag="wh_f32")
            nc.sync.dma_start(out=wh_f32, in_=wh.rearrange("(c p) n -> p c n", p=P))
            wh8 = consts.tile([P, KC, D], fp8, tag="wh8")
            nc.scalar.mul(out=wh8, in_=wh_f32, mul=WSCALE)

            wa_sb = consts.tile([P, KC, D], f32, tag="wa")
            nc.sync.dma_start(out=wa_sb, in_=wa.rearrange("(c p) n -> p c n", p=P))

            va_sb = consts.tile([P, NTB, TBLK], f32, tag="va")
            va_bcast = bass.AP(
                tensor=va.tensor, offset=va.offset, ap=[[0, P], va.ap[-1]]
            )
            nc.gpsimd.dma_start(
                out=va_sb.rearrange("p a b -> p (a b)"), in_=va_bcast
            )

            ones_col = consts.tile([P, 1], f32, tag="ones_col")
            nc.vector.memset(ones_col, 1.0)

            # ---------------- phase 0: s1T = (x0 @ W_a)^T ----------------
            x0_nat = small.tile([P, D], f32, tag="x0_nat")
            nc.vector.memset(x0_nat, 0.0)
            nc.sync.dma_start(out=x0_nat[:BL, :], in_=x0s)
            x0t_sb = small.tile([P, KC, BL], f32, tag="x0t")
            for k in range(KC):
                x0t_ps = misc_ps.tile([P, P], f32, tag="misc")
                nc.tensor.transpose(
                    x0t_ps, x0_nat[:, k * P : (k + 1) * P], identf
                )
                nc.vector.tensor_copy(out=x0t_sb[:, k, :], in_=x0t_ps[:, :BL])

            s1t_sb = small.tile([P, NCH, BL], f32, tag="s1t")
            for n in range(NCH):
                s1_ps = misc_ps.tile([P, BL], f32, tag="misc")
                for k in range(KC):
                    nc.tensor.matmul(
                        s1_ps,
                        lhsT=wa_sb[:, k, n * P : (n + 1) * P],
                        rhs=x0t_sb[:, k, :],
                        start=(k == 0),
                        stop=(k == KC - 1),
                    )
                nc.vector.tensor_copy(out=s1t_sb[:, n, :], in_=s1_ps)

            # ---------------- main loop ----------------
            o_parts = small.tile([P, NCH * BL], f32, tag="o_parts")
            xsums = small.tile([P, 2 * BL * NTB], f32, tag="xsums")

            for b in range(BL):
                s_sb = s_pool.tile([P, NCH, NTB, TBLK], f32, tag="s")
                for tb in range(NTB):
                    it = b * NTB + tb
                    if it < NPRE:
                        nat = prenat[it]
                    else:
                        nat = nat_pool.tile([P, SUB, D], bf16, tag="nat")
                        src = x1s[b, tb * TBLK : (tb + 1) * TBLK, :].rearrange(
                            "(s p) d -> p s d", p=P
                        )
                        nc.gpsimd.dma_start(out=nat, in_=src)  # SWDGE cast

                    # x1 tile -> transposed k-chunks, split 3+1 across two
                    # psum tiles so the copies start early and free them fast
                    xt = xt_pool.tile([P, KC, TBLK], fp8, tag="xt")
                    tpa = tpa_pool.tile([P, 3, TBLK], f32, tag="tpa")
                    for dk in range(3):
                        for s in range(SUB):
                            nc.tensor.matmul(
                                tpa[:, dk, s * P : (s + 1) * P],
                                lhsT=nat[:, s, dk * P : (dk + 1) * P],
                                rhs=ident,
                                start=True,
                                stop=True,
                            )
                    # DVE: chunks 0-2 -> fp8 (+ x1 partial sums via accum)
                    nc.vector.tensor_scalar(
                        out=xt[:, 0:3, :],
                        in0=tpa,
                        scalar1=0.0,
                        scalar2=0.0,
                        op0=mybir.AluOpType.add,
                        op1=mybir.AluOpType.add,
                        accum_out=xsums[:, 2 * it : 2 * it + 1],
                    )
                    tpb = tpb_pool.tile([P, 1, TBLK], f32, tag="tpb")
                    for s in range(SUB):
                        nc.tensor.matmul(
                            tpb[:, 0, s * P : (s + 1) * P],
                            lhsT=nat[:, s, 3 * P : 4 * P],
                            rhs=ident,
                            start=True,
                            stop=True,
                        )
                    # ACT: chunk 3 -> fp8 (+ accum)
                    nc.scalar.activation(
                        out=xt[:, 3, :],
                        in_=tpb[:, 0, :],
                        func=mybir.ActivationFunctionType.Copy,
                        accum_out=xsums[:, 2 * it + 1 : 2 * it + 2],
                    )

                    # main matmul: fp8 DoubleRow, 2 MMs of K=256 per n-chunk
                    for n in range(NCH):
                        mm = mm_pool.tile([P, TBLK], f32, tag="mm")
                        for q in range(2):
                            nc.tensor.matmul(
                                mm,
                                lhsT=wh8[:, 2 * q : 2 * q + 2, n * P : (n + 1) * P],
                                rhs=xt[:, 2 * q : 2 * q + 2, :],
                                start=(q == 0),
                                stop=(q == 1),
                                perf_mode=mybir.MatmulPerfMode.DoubleRow,
                            )
                        nc.scalar.activation(
                            out=s_sb[:, n, tb, :],
                            in_=mm,
                            func=mybir.ActivationFunctionType.Tanh,
                            bias=s1t_sb[:, n, b : b + 1],
                            scale=1.0 / WSCALE,
                        )

                # V_a-weighted t-sums over the whole batch (FD=2048)
                for n in range(NCH):
                    scr = scr_pool.tile([P, NTB, TBLK], f32, tag="scr")
                    idx = n * BL + b
                    nc.vector.scalar_tensor_tensor(
                        out=scr,
                        in0=s_sb[:, n, :, :],
                        scalar=1.0,
                        in1=va_sb,
                        op0=mybir.AluOpType.mult,
                        op1=mybir.AluOpType.mult,
                        accum_out=o_parts[:, idx : idx + 1],
                    )

            # ---------------- epilogue ----------------
            # transpose o -> [b, d]
            o3 = o_parts.rearrange("p (n b) -> p n b", b=BL)
            ot_ps = misc_ps.tile([BL, D], f32, tag="misc")
            for n in range(NCH):
                nc.tensor.transpose(
                    ot_ps[:, n * P : (n + 1) * P], o3[:, n, :], identf
                )
            ot_sb = small.tile([BL, D], f32, tag="ot_sb")
            nc.vector.tensor_copy(out=ot_sb, in_=ot_ps)

            # sum(x1[b]): reduce per-batch accum columns, then contract d
            xb_sb = small.tile([P, BL], f32, tag="xb_sb")
            nc.vector.reduce_sum(
                out=xb_sb,
                in_=xsums.rearrange("p (b q) -> p b q", q=2 * NTB),
                axis=mybir.AxisListType.X,
            )
            sx1_ps = misc_ps.tile([BL, 1], f32, tag="misc")
            nc.tensor.matmul(sx1_ps, lhsT=xb_sb, rhs=ones_col, start=True, stop=True)
            sx1 = small.tile([BL, 1], f32, tag="sx1")
            nc.vector.tensor_copy(out=sx1, in_=sx1_ps)

            # softmax over D, then scale by sum(x1)
            neg_max = small.tile([BL, 1], f32, tag="neg_max")
            nc.vector.reduce_max(
                out=neg_max, in_=ot_sb, axis=mybir.AxisListType.X, negate=True
            )
            exp_sb = small.tile([BL, D], f32, tag="exp_sb")
            sum_exp = small.tile([BL, 1], f32, tag="sum_exp")
            nc.scalar.activation(
                out=exp_sb,
                in_=ot_sb,
                func=mybir.ActivationFunctionType.Exp,
                bias=neg_max,
                accum_out=sum_exp,
            )
            rec = small.tile([BL, 1], f32, tag="rec")
            nc.vector.reciprocal(out=rec, in_=sum_exp)
            scale = small.tile([BL, 1], f32, tag="scale")
            nc.vector.tensor_mul(out=scale, in0=rec, in1=sx1)
            out_sb = small.tile([BL, D], f32, tag="out_sb")
            nc.vector.tensor_scalar_mul(out=out_sb, in0=exp_sb, scalar1=scale)
            nc.sync.dma_start(out=out, in_=out_sb)

    nc.finalize()
    return nc


def make_in_maps(x0, x1, W_a, W_h, V_a):
    x0 = np.ascontiguousarray(x0, dtype=np.float32)
    x1 = np.ascontiguousarray(x1, dtype=np.float32)
    W_a = np.ascontiguousarray(W_a, dtype=np.float32)
    W_h = np.ascontiguousarray(W_h, dtype=np.float32)
    V_a = np.ascontiguousarray(V_a, dtype=np.float32)
    in_maps = []
    for c in range(NCORES):
        sl = slice(c * BL, (c + 1) * BL)
        in_maps.append(
            {
                "x0s": np.ascontiguousarray(x0[sl]),
                "x1s": np.ascontiguousarray(x1[sl]),
                "W_a": W_a,
                "W_h": W_h,
                "V_a": V_a,
            }
        )
    return in_maps


_NC_CACHE = {}


def _build(mode):
    if mode == "v2":
        return build_nc_v2()
    if mode == "a":
        return build_nc_a()
    return build_nc(mode)


def kernel(x0, x1, W_a, W_h, V_a):
    from concourse.bass_utils import run_bass_kernel_spmd

    mode = "v2"
    nc = _NC_CACHE.get(mode)
    if nc is None:
        nc = _NC_CACHE[mode] = _build(mode)
    in_maps = make_in_maps(x0, x1, W_a, W_h, V_a)
    res = run_bass_kernel_spmd(nc, in_maps, core_ids=list(range(NCORES)))
    return np.concatenate([res.results[c]["out"] for c in range(NCORES)], axis=0)



# revision 9
# speedup vs baseline: 1.4874x; 1.2742x over previous
"""Trainium2 Bass kernel for nn_AttentionLayer (B=32, T=2048, D=512).

Computation (per batch b):
    s1 = x0 @ W_a                       # (D,)
    s2 = x1[b] @ W_h                    # (T, D)
    s  = tanh(s1 + s2)                  # (T, D)
    o  = V_a @ s                        # (D,)   (contract T)
    alpha = softmax(o)                  # over D
    out[b] = alpha * sum(x1[b])

Sharding: data-parallel over batch across 8 NeuronCores (4 batches/core),
weights replicated, no cross-core communication.

Per-core dataflow, default mode "bf16" (measured 128 us on TRN2, rel err
~2e-3; mode "f32" is the full-precision fallback at ~150 us, rel err ~5e-5):
  - SWDGE cast-DMA x1 natural tile [128t, 4s, 512d] fp32->bf16 into SBUF
  - x1 -> x1T chunks [128d, 512t] via REGULAR bf16 matmuls against an
    identity (1 cyc/row, keeps the PE HAM warm; transpose-mode measures
    ~350 ns/op and regular fp32 matmuls are 4 cyc/row)
  - PSUM->SBUF copies of x1T split across ACT/DVE; their accum_out emits
    per-partition partial sums of x1 for free (-> sum(x1[b]) later)
  - PE matmul (bf16): s2T[dout, t] = sum_k W_h[k, dout] * x1T[k, t]
  - ACT: tanh(s2T + bias s1T[dout, 1]) fused, PSUM -> SBUF
  - DVE scalar_tensor_tensor with partition-broadcast V_a: weighted t-sums
    (InstTensorTensorReduce and all-bf16 ScalarTensorTensor+accum both
    wedge TRN2; fp32 ScalarTensorTensor is the working form)
  - Epilogue: softmax over D on [4, 512] rows, scale by sum(x1[b])
"""

import numpy as np

B, T, D = 32, 2048, 512
NCORES = 8
BL = B // NCORES          # batches per core
P = 128
KC = D // P               # k chunks (4)
NCH = D // P              # output-d chunks (4)
TBLK = 512                # t elements per block
NTB = T // TBLK           # t blocks (4)
SUB = TBLK // P           # natural subtiles per block (4)


def build_nc(mode: str = "f32"):
    """Build the per-core Bass program. mode in {"f32", "bf16"}."""
    import concourse.bass as bass
    import concourse.tile as tile
    from concourse import bacc, mybir
    from concourse.masks import make_identity

    f32 = mybir.dt.float32
    f32r = mybir.dt.float32r
    bf16 = mybir.dt.bfloat16
    dt_nat = f32 if mode == "f32" else bf16     # natural x1 tiles
    dt_mm = f32r if mode == "f32" else bf16     # matmul operand tiles
    # tanh output / va / scr stay fp32: the bf16 ScalarTensorTensor+accum
    # path wedges TRN2 hardware (perf-mode + accumulator readout)
    dt_s = f32

    nc = bacc.Bacc("TRN2", target_bir_lowering=False)

    x0s = nc.dram_tensor("x0s", [BL, D], f32, kind="ExternalInput").ap()
    x1s = nc.dram_tensor("x1s", [BL, T, D], f32, kind="ExternalInput").ap()
    wa = nc.dram_tensor("W_a", [D, D], f32, kind="ExternalInput").ap()
    wh = nc.dram_tensor("W_h", [D, D], f32, kind="ExternalInput").ap()
    va = nc.dram_tensor("V_a", [1, T], f32, kind="ExternalInput").ap()
    out = nc.dram_tensor("out", [BL, D], f32, kind="ExternalOutput").ap()

    with tile.TileContext(nc) as tc:
        with (
            tc.tile_pool(name="consts", bufs=1) as consts,
            tc.tile_pool(name="nat", bufs=3) as nat_pool,
            tc.tile_pool(name="xt", bufs=8) as xt_pool,
            tc.tile_pool(name="s", bufs=4) as s_pool,
            tc.tile_pool(name="scr", bufs=2) as scr_pool,
            tc.tile_pool(name="small", bufs=1) as small,
            tc.tile_pool(name="misc_ps", bufs=1, space="PSUM") as misc_ps,
            tc.tile_pool(name="tp_ps", bufs=2, space="PSUM") as tp_pool,
            tc.tile_pool(name="mm_ps", bufs=4, space="PSUM") as mm_pool,
        ):
            # ---------------- constants ----------------
            ident = consts.tile([P, P], dt_nat, tag="ident")
            make_identity(nc, ident)
            identf = ident
            if dt_nat != f32:
                identf = consts.tile([P, P], f32, tag="identf")
                make_identity(nc, identf)

            # W_h in matmul dtype (f32r needs an explicit rounding producer)
            wh_sb = consts.tile([P, KC, D], dt_mm, tag="wh")
            if mode == "f32":
                wh_f32 = consts.tile([P, KC, D], f32, tag="wh_f32")
                nc.sync.dma_start(
                    out=wh_f32, in_=wh.rearrange("(c p) n -> p c n", p=P)
                )
                nc.vector.tensor_copy(out=wh_sb, in_=wh_f32)
            else:
                nc.gpsimd.dma_start(
                    out=wh_sb, in_=wh.rearrange("(c p) n -> p c n", p=P)
                )
            wa_sb = consts.tile([P, KC, D], f32, tag="wa")
            nc.sync.dma_start(out=wa_sb, in_=wa.rearrange("(c p) n -> p c n", p=P))

            va_sb = consts.tile([P, T], dt_s, tag="va")
            va_bcast = bass.AP(
                tensor=va.tensor, offset=va.offset, ap=[[0, P], va.ap[-1]]
            )
            nc.gpsimd.dma_start(out=va_sb, in_=va_bcast)

            # all-ones column for the final cross-partition sum
            ones_col = consts.tile([P, 1], f32, tag="ones_col")
            nc.vector.memset(ones_col, 1.0)

            # ---------------- phase 0: s1T = (x0 @ W_a)^T ----------------
            # pad x0 to 128 partitions: contraction dims < 128 are flaky on HW
            x0_nat = small.tile([P, D], f32, tag="x0_nat")
            nc.vector.memset(x0_nat, 0.0)
            nc.sync.dma_start(out=x0_nat[:BL, :], in_=x0s)
            x0t_sb = small.tile([P, KC, BL], f32, tag="x0t")
            for k in range(KC):
                x0t_ps = misc_ps.tile([P, P], f32, tag="misc")
                nc.tensor.transpose(
                    x0t_ps, x0_nat[:, k * P : (k + 1) * P], identf
                )
                nc.vector.tensor_copy(out=x0t_sb[:, k, :], in_=x0t_ps[:, :BL])

            s1t_sb = small.tile([P, NCH, BL], f32, tag="s1t")
            for n in range(NCH):
                s1_ps = misc_ps.tile([P, BL], f32, tag="misc")
                for k in range(KC):
                    nc.tensor.matmul(
                        s1_ps,
                        lhsT=wa_sb[:, k, n * P : (n + 1) * P],
                        rhs=x0t_sb[:, k, :],
                        start=(k == 0),
                        stop=(k == KC - 1),
                    )
                nc.vector.tensor_copy(out=s1t_sb[:, n, :], in_=s1_ps)

            # ---------------- main loop ----------------
            o_parts = small.tile([P, NCH * BL * NTB], f32, tag="o_parts")
            # per-(b,tb,dk) partial sums of x1 (per d-partition), free via the
            # accum_out of the transpose-copy ops
            xsums = small.tile([P, BL * NTB * KC], f32, tag="xsums")

            for b in range(BL):
                for tb in range(NTB):
                    nat = nat_pool.tile([P, SUB, D], dt_nat, tag="nat")
                    src = x1s[b, tb * TBLK : (tb + 1) * TBLK, :].rearrange(
                        "(s p) d -> p s d", p=P
                    )
                    if dt_nat == f32:
                        nc.sync.dma_start(out=nat, in_=src)
                    else:
                        nc.gpsimd.dma_start(out=nat, in_=src)  # SWDGE cast

                    # transpose x1 tile: [t, d] -> 4 chunks [128d, 512t];
                    # the PSUM->SBUF copies also emit per-partition sums.
                    # bf16: regular matmuls vs identity (1 cyc/row, keeps HAM
                    # warm) into fp32 PSUM; f32: transpose-mode.
                    xts = []
                    for dk in range(KC):
                        if dt_nat == bf16:
                            tp_ps = tp_pool.tile([P, TBLK], f32, tag="tp")
                            for s in range(SUB):
                                nc.tensor.matmul(
                                    tp_ps[:, s * P : (s + 1) * P],
                                    lhsT=nat[:, s, dk * P : (dk + 1) * P],
                                    rhs=ident,
                                    start=True,
                                    stop=True,
                                )
                        else:
                            tp_ps = tp_pool.tile([P, TBLK], dt_nat, tag="tp")
                            for s in range(SUB):
                                nc.tensor.transpose(
                                    tp_ps[:, s * P : (s + 1) * P],
                                    nat[:, s, dk * P : (dk + 1) * P],
                                    ident,
                                )
                        xt = xt_pool.tile([P, TBLK], dt_mm, tag="xt")
                        xi = (b * NTB + tb) * KC + dk
                        if dk < 2:
                            nc.scalar.activation(
                                out=xt,
                                in_=tp_ps,
                                func=mybir.ActivationFunctionType.Copy,
                                accum_out=xsums[:, xi : xi + 1],
                            )
                        else:
                            nc.vector.tensor_scalar(
                                out=xt,
                                in0=tp_ps,
                                scalar1=0.0,
                                scalar2=0.0,
                                op0=mybir.AluOpType.add,
                                op1=mybir.AluOpType.add,
                                accum_out=xsums[:, xi : xi + 1],
                            )
                        xts.append(xt)

                    # s2T = W_h^T x1^T ; tanh(+s1T bias) ; V_a-weighted t-sum
                    for n in range(NCH):
                        mm_ps = mm_pool.tile([P, TBLK], f32, tag="mm")
                        for dk in range(KC):
                            nc.tensor.matmul(
                                mm_ps,
                                lhsT=wh_sb[:, dk, n * P : (n + 1) * P],
                                rhs=xts[dk][:],
                                start=(dk == 0),
                                stop=(dk == KC - 1),
                            )
                        s_sb = s_pool.tile([P, TBLK], dt_s, tag="s")
                        nc.scalar.activation(
                            out=s_sb,
                            in_=mm_ps,
                            func=mybir.ActivationFunctionType.Tanh,
                            bias=s1t_sb[:, n, b : b + 1],
                        )
                        scr = scr_pool.tile([P, TBLK], dt_s, tag="scr")
                        idx = (n * BL + b) * NTB + tb
                        # out = (s * 1.0) * va ; accum_out = sum(out) per
                        # partition. (InstTensorTensorReduce wedges TRN2 here;
                        # InstScalarTensorTensor is the working equivalent.)
                        nc.vector.scalar_tensor_tensor(
                            out=scr,
                            in0=s_sb,
                            scalar=1.0,
                            in1=va_sb[:, tb * TBLK : (tb + 1) * TBLK],
                            op0=mybir.AluOpType.mult,
                            op1=mybir.AluOpType.mult,
                            accum_out=o_parts[:, idx : idx + 1],
                        )

            # ---------------- epilogue ----------------
            # o (transposed layout) = sum of partials over t-blocks
            o_sb = small.tile([P, NCH * BL], f32, tag="o_sb")
            nc.vector.reduce_sum(
                out=o_sb,
                in_=o_parts.rearrange("p (q t) -> p q t", t=NTB),
                axis=mybir.AxisListType.X,
            )
            # transpose o -> [b, d]
            o3 = o_sb.rearrange("p (n b) -> p n b", b=BL)
            ot_ps = misc_ps.tile([BL, D], f32, tag="misc")
            for n in range(NCH):
                nc.tensor.transpose(
                    ot_ps[:, n * P : (n + 1) * P], o3[:, n, :], identf
                )
            ot_sb = small.tile([BL, D], f32, tag="ot_sb")
            nc.vector.tensor_copy(out=ot_sb, in_=ot_ps)

            # sum(x1[b]) finish: reduce per-batch columns, then contract the
            # partition (d) axis with a ones-vector matmul -> [BL, 1]
            xb_sb = small.tile([P, BL], f32, tag="xb_sb")
            nc.vector.reduce_sum(
                out=xb_sb,
                in_=xsums.rearrange("p (b q) -> p b q", q=NTB * KC),
                axis=mybir.AxisListType.X,
            )
            sx1_ps = misc_ps.tile([BL, 1], f32, tag="misc")
            nc.tensor.matmul(sx1_ps, lhsT=xb_sb, rhs=ones_col, start=True, stop=True)
            sx1 = small.tile([BL, 1], f32, tag="sx1")
            nc.vector.tensor_copy(out=sx1, in_=sx1_ps)

            # softmax over D, then scale by sum(x1)
            neg_max = small.tile([BL, 1], f32, tag="neg_max")
            nc.vector.reduce_max(
                out=neg_max, in_=ot_sb, axis=mybir.AxisListType.X, negate=True
            )
            exp_sb = small.tile([BL, D], f32, tag="exp_sb")
            sum_exp = small.tile([BL, 1], f32, tag="sum_exp")
            nc.scalar.activation(
                out=exp_sb,
                in_=ot_sb,
                func=mybir.ActivationFunctionType.Exp,
                bias=neg_max,
                accum_out=sum_exp,
            )
            rec = small.tile([BL, 1], f32, tag="rec")
            nc.vector.reciprocal(out=rec, in_=sum_exp)
            scale = small.tile([BL, 1], f32, tag="scale")
            nc.vector.tensor_mul(out=scale, in0=rec, in1=sx1)
            out_sb = small.tile([BL, D], f32, tag="out_sb")
            nc.vector.tensor_scalar_mul(out=out_sb, in0=exp_sb, scalar1=scale)
            nc.sync.dma_start(out=out, in_=out_sb)

    nc.finalize()
    return nc


def build_nc_a():
    """Orientation-A bf16 build: s2 tiles in [t, d_out] layout.

    Per (batch, 512-t-block) iteration:
      - SWDGE cast-load x1 tile [128t, 4s, 512d] fp32->bf16
      - x1T chunks via REGULAR bf16 matmuls against identity (N=128 pipelined
        rate; transpose-mode would cost ~350ns/op and skip HAM warmup)
      - two [128, 1024]-bf16 one-bank PSUM tiles -> two DVE copies with
        accum_out (x1 partial sums ride along for free)
      - s2 psum [t=128, 512dout] = sum_dk x1T_chunk.T @ W_h[dk] plus a 5th
        rank-1 accumulation ones.T @ (s1[b]/128 replicated) folding in the
        tanh bias
      - ACT: plain tanh psum -> SBUF bf16
      - V_a contraction on PE: psum_o[1, 512] += va_col.T @ s_tile,
        accumulated over a batch's 16 t-chunks
    Epilogue: softmax on [4, 512] rows + scale by sum(x1).
    """
    import concourse.bass as bass
    import concourse.tile as tile
    from concourse import bacc, mybir
    from concourse.masks import make_identity

    f32 = mybir.dt.float32
    bf16 = mybir.dt.bfloat16

    nc = bacc.Bacc("TRN2", target_bir_lowering=False)

    x0s = nc.dram_tensor("x0s", [BL, D], f32, kind="ExternalInput").ap()
    x1s = nc.dram_tensor("x1s", [BL, T, D], f32, kind="ExternalInput").ap()
    wa = nc.dram_tensor("W_a", [D, D], f32, kind="ExternalInput").ap()
    wh = nc.dram_tensor("W_h", [D, D], f32, kind="ExternalInput").ap()
    va = nc.dram_tensor("V_a", [1, T], f32, kind="ExternalInput").ap()
    out = nc.dram_tensor("out", [BL, D], f32, kind="ExternalOutput").ap()

    with tile.TileContext(nc) as tc:
        with (
            tc.tile_pool(name="consts", bufs=1) as consts,
            tc.tile_pool(name="nat", bufs=3) as nat_pool,
            tc.tile_pool(name="xt", bufs=4) as xt_pool,
            tc.tile_pool(name="s", bufs=4) as s_pool,
            tc.tile_pool(name="small", bufs=1) as small,
            tc.tile_pool(name="dram", bufs=1, space="DRAM") as dram_pool,
            tc.tile_pool(name="misc_ps", bufs=1, space="PSUM") as misc_ps,
            tc.tile_pool(name="tp_ps", bufs=2, space="PSUM") as tp_pool,
            tc.tile_pool(name="mm_ps", bufs=4, space="PSUM") as mm_pool,
            tc.tile_pool(name="o_ps", bufs=1, space="PSUM") as o_pool,
        ):
            # ---------------- constants ----------------
            ident = consts.tile([P, P], bf16, tag="ident")
            make_identity(nc, ident)
            identf = consts.tile([P, P], f32, tag="identf")
            make_identity(nc, identf)
            ones_bf = consts.tile([P, P], bf16, tag="ones_bf")
            nc.vector.memset(ones_bf, 1.0)
            ones_col = consts.tile([P, 1], f32, tag="ones_col")
            nc.vector.memset(ones_col, 1.0)

            wh_sb = consts.tile([P, KC, D], bf16, tag="wh")
            nc.gpsimd.dma_start(out=wh_sb, in_=wh.rearrange("(c p) n -> p c n", p=P))
            wa_sb = consts.tile([P, KC, D], f32, tag="wa")
            nc.sync.dma_start(out=wa_sb, in_=wa.rearrange("(c p) n -> p c n", p=P))

            # va columns: va_sb[p, c] = V_a[c*128 + p]
            va_sb = consts.tile([P, T // P], bf16, tag="va")
            nc.gpsimd.dma_start(
                out=va_sb, in_=va.rearrange("a (c p) -> p (a c)", p=P)
            )
            # va_ind[:, b, c, b'] = va column c if b' == b else 0: batch b's
            # V_a matmuls route their sums into psum row b only
            va_ind = consts.tile([P, BL, T // P, BL], bf16, tag="va_ind")
            nc.vector.memset(va_ind, 0.0)
            for b in range(BL):
                nc.vector.tensor_copy(out=va_ind[:, b, :, b], in_=va_sb)

            # ---------------- phase 0: s1/128 rows + broadcast ----------
            x0_nat = small.tile([P, D], f32, tag="x0_nat")
            nc.vector.memset(x0_nat, 0.0)
            nc.sync.dma_start(out=x0_nat[:BL, :], in_=x0s)
            nc.scalar.mul(out=x0_nat, in_=x0_nat, mul=1.0 / P)
            x0t_sb = small.tile([P, KC, BL], f32, tag="x0t")
            for k in range(KC):
                x0t_ps = misc_ps.tile([P, P], f32, tag="misc")
                nc.tensor.transpose(x0t_ps, x0_nat[:, k * P : (k + 1) * P], identf)
                nc.vector.tensor_copy(out=x0t_sb[:, k, :], in_=x0t_ps[:, :BL])

            s1_ps = misc_ps.tile([BL, D], f32, tag="misc")
            for k in range(KC):
                nc.tensor.matmul(
                    s1_ps,
                    lhsT=x0t_sb[:, k, :],
                    rhs=wa_sb[:, k, :],
                    start=(k == 0),
                    stop=(k == KC - 1),
                )
            s1_row = small.tile([BL, D], f32, tag="s1_row")
            nc.vector.tensor_copy(out=s1_row, in_=s1_ps)
            s1_dram = dram_pool.tile([BL, D], f32, tag="s1_dram")
            nc.sync.dma_start(out=s1_dram, in_=s1_row)
            s1rep = consts.tile([P, BL, D], bf16, tag="s1rep")
            for b in range(BL):
                row = s1_dram[b : b + 1, :]
                bcast = bass.AP(
                    tensor=row.tensor, offset=row.offset, ap=[[0, P], row.ap[-1]]
                )
                nc.gpsimd.dma_start(out=s1rep[:, b, :], in_=bcast)

            # ---------------- main loop ----------------
            xsums = small.tile([P, BL * NTB * KC], f32, tag="xsums")
            ot_sb = small.tile([BL, D], f32, tag="ot_sb")
            o_ps = o_pool.tile([BL, D], f32, tag="o")

            for b in range(BL):
                for tb in range(NTB):
                    nat = nat_pool.tile([P, SUB, D], bf16, tag="nat")
                    src = x1s[b, tb * TBLK : (tb + 1) * TBLK, :].rearrange(
                        "(s p) d -> p s d", p=P
                    )
                    nc.gpsimd.dma_start(out=nat, in_=src)  # SWDGE cast

                    # x1T chunks as regular matmuls vs identity (fp32 PSUM out)
                    xts = []
                    for dk in range(KC):
                        tp_ps = tp_pool.tile([P, TBLK], f32, tag="tp")
                        for s in range(SUB):
                            nc.tensor.matmul(
                                tp_ps[:, s * P : (s + 1) * P],
                                lhsT=nat[:, s, dk * P : (dk + 1) * P],
                                rhs=ident,
                                start=True,
                                stop=True,
                            )
                        xt = xt_pool.tile([P, TBLK], bf16, tag="xt")
                        xi = (b * NTB + tb) * KC + dk
                        nc.vector.tensor_scalar(
                            out=xt,
                            in0=tp_ps,
                            scalar1=0.0,
                            scalar2=0.0,
                            op0=mybir.AluOpType.add,
                            op1=mybir.AluOpType.add,
                            accum_out=xsums[:, xi : xi + 1],
                        )
                        xts.append(xt)

                    # s2 tiles [t=128, dout=512] + rank-1 s1 bias, tanh, V_a
                    for ts_ in range(SUB):
                        mm_ps = mm_pool.tile([P, TBLK], f32, tag="mm")
                        for dk in range(KC):
                            nc.tensor.matmul(
                                mm_ps,
                                lhsT=xts[dk][:, ts_ * P : (ts_ + 1) * P],
                                rhs=wh_sb[:, dk, :],
                                start=(dk == 0),
                                stop=False,
                            )
                        nc.tensor.matmul(
                            mm_ps,
                            lhsT=ones_bf,
                            rhs=s1rep[:, b, :],
                            start=False,
                            stop=True,
                        )
                        s_sb = s_pool.tile([P, TBLK], bf16, tag="s")
                        nc.scalar.activation(
                            out=s_sb,
                            in_=mm_ps,
                            func=mybir.ActivationFunctionType.Tanh,
                        )
                        c = tb * SUB + ts_
                        nc.tensor.matmul(
                            o_ps,
                            lhsT=va_ind[:, b, c, :],
                            rhs=s_sb,
                            start=(b == 0 and c == 0),
                            stop=(b == BL - 1 and c == NTB * SUB - 1),
                            skip_group_check=True,
                        )
            nc.vector.tensor_copy(out=ot_sb, in_=o_ps)

            # ---------------- epilogue ----------------
            xb_sb = small.tile([P, BL], f32, tag="xb_sb")
            nc.vector.reduce_sum(
                out=xb_sb,
                in_=xsums.rearrange("p (b q) -> p b q", q=NTB * KC),
                axis=mybir.AxisListType.X,
            )
            sx1_ps = misc_ps.tile([BL, 1], f32, tag="misc")
            nc.tensor.matmul(sx1_ps, lhsT=xb_sb, rhs=ones_col, start=True, stop=True)
            sx1 = small.tile([BL, 1], f32, tag="sx1")
            nc.vector.tensor_copy(out=sx1, in_=sx1_ps)

            neg_max = small.tile([BL, 1], f32, tag="neg_max")
            nc.vector.reduce_max(
                out=neg_max, in_=ot_sb, axis=mybir.AxisListType.X, negate=True
            )
            exp_sb = small.tile([BL, D], f32, tag="exp_sb")
            sum_exp = small.tile([BL, 1], f32, tag="sum_exp")
            nc.scalar.activation(
                out=exp_sb,
                in_=ot_sb,
                func=mybir.ActivationFunctionType.Exp,
                bias=neg_max,
                accum_out=sum_exp,
            )
            rec = small.tile([BL, 1], f32, tag="rec")
            nc.vector.reciprocal(out=rec, in_=sum_exp)
            scale = small.tile([BL, 1], f32, tag="scale")
            nc.vector.tensor_mul(out=scale, in0=rec, in1=sx1)
            out_sb = small.tile([BL, D], f32, tag="out_sb")
            nc.vector.tensor_scalar_mul(out=out_sb, in0=exp_sb, scalar1=scale)
            nc.sync.dma_start(out=out, in_=out_sb)

    nc.finalize()
    return nc


def build_nc_v2():
    """fp8 DoubleRow main matmul + batched V_a reduction.

    Per (batch, 512-t-block) iteration:
      - SWDGE cast-DMA x1 tile [128t, 4s, 512d] fp32->bf16
      - 16 transpose-matmuls (bf16 vs identity) into ONE psum tile
        tp [128d, 4dk, 512t] (4 banks)
      - copies tp -> xt fp8e4 [128, 4dk, 512]: DVE takes chunks 0-2,
        ACT takes chunk 3; both with accum_out -> bf16-accurate x1 sums
      - main matmul in fp8 DoubleRow: per n-chunk psum [128dout, 512t]
        accumulates 2 MMs, each contracting 256 k (2 k-tiles paired);
        W_h pre-scaled by 64 into fp8 (values ~1.3 rms, safe in e4m3)
      - ACT: tanh(psum * 1/64 + s1T bias) -> s_sb f32 [128, n, tb, 512]
      - after each batch's 4 t-blocks: 4 DVE scalar_tensor_tensor ops of
        FD=2048 (s * va, accum per partition) -> o_parts column
    Epilogue: softmax over D + scale by sum(x1), as baseline.
    """
    import concourse.bass as bass
    import concourse.tile as tile
    from concourse import bacc, mybir
    from concourse.masks import make_identity

    f32 = mybir.dt.float32
    bf16 = mybir.dt.bfloat16
    fp8 = mybir.dt.float8e4
    WSCALE = 64.0

    nc = bacc.Bacc("TRN2", target_bir_lowering=False)

    x0s = nc.dram_tensor("x0s", [BL, D], f32, kind="ExternalInput").ap()
    x1s = nc.dram_tensor("x1s", [BL, T, D], f32, kind="ExternalInput").ap()
    wa = nc.dram_tensor("W_a", [D, D], f32, kind="ExternalInput").ap()
    wh = nc.dram_tensor("W_h", [D, D], f32, kind="ExternalInput").ap()
    va = nc.dram_tensor("V_a", [1, T], f32, kind="ExternalInput").ap()
    out = nc.dram_tensor("out", [BL, D], f32, kind="ExternalOutput").ap()

    with tile.TileContext(nc) as tc:
        with (
            tc.tile_pool(name="consts", bufs=1) as consts,
            tc.tile_pool(name="nat", bufs=4) as nat_pool,
            tc.tile_pool(name="xt", bufs=3) as xt_pool,
            tc.tile_pool(name="s", bufs=2) as s_pool,
            tc.tile_pool(name="scr", bufs=2) as scr_pool,
            tc.tile_pool(name="small", bufs=1) as small,
            tc.tile_pool(name="misc_ps", bufs=1, space="PSUM") as misc_ps,
            tc.tile_pool(name="tpa_ps", bufs=1, space="PSUM") as tpa_pool,
            tc.tile_pool(name="tpb_ps", bufs=1, space="PSUM") as tpb_pool,
            tc.tile_pool(name="mm_ps", bufs=3, space="PSUM") as mm_pool,
        ):
            # ---------------- prefetch x1 tiles, then constants ----------
            # Issue the first nat DMAs before anything else lands on the
            # SWDGE queue so the PE can start transposing ASAP.
            NPRE = 3
            prenat = []
            for i in range(NPRE):
                b, tb = divmod(i, NTB)
                nat = nat_pool.tile([P, SUB, D], bf16, tag="nat")
                src = x1s[b, tb * TBLK : (tb + 1) * TBLK, :].rearrange(
                    "(s p) d -> p s d", p=P
                )
                if i == 0:
                    # split the first tile into per-subtile DMAs so the PE
                    # can start transposing ~3 transfers earlier
                    for s in range(SUB):
                        nc.gpsimd.dma_start(
                            out=nat[:, s, :],
                            in_=x1s[
                                b, tb * TBLK + s * P : tb * TBLK + (s + 1) * P, :
                            ].rearrange("p d -> p d"),
                        )
                else:
                    nc.gpsimd.dma_start(out=nat, in_=src)
                prenat.append(nat)

            ident = consts.tile([P, P], bf16, tag="ident")
            make_identity(nc, ident)
            identf = consts.tile([P, P], f32, tag="identf")
            make_identity(nc, identf)

            # W_h * 64 in fp8e4, k-chunk layout [128, kc, dout]
            wh_f32 = consts.tile([P, KC, D], f32, tag="wh_f32")
            nc.sync.dma_start(out=wh_f32, in_=wh.rearrange("(c p) n -> p c n", p=P))
            wh8 = consts.tile([P, KC, D], fp8, tag="wh8")
            nc.scalar.mul(out=wh8, in_=wh_f32, mul=WSCALE)

            wa_sb = consts.tile([P, KC, D], f32, tag="wa")
            nc.sync.dma_start(out=wa_sb, in_=wa.rearrange("(c p) n -> p c n", p=P))

            va_sb = consts.tile([P, NTB, TBLK], f32, tag="va")
            va_bcast = bass.AP(
                tensor=va.tensor, offset=va.offset, ap=[[0, P], va.ap[-1]]
            )
            nc.gpsimd.dma_start(
                out=va_sb.rearrange("p a b -> p (a b)"), in_=va_bcast
            )

            ones_col = consts.tile([P, 1], f32, tag="ones_col")
            nc.vector.memset(ones_col, 1.0)

            # ---------------- phase 0: s1T = (x0 @ W_a)^T ----------------
            x0_nat = small.tile([P, D], f32, tag="x0_nat")
            nc.vector.memset(x0_nat, 0.0)
            nc.sync.dma_start(out=x0_nat[:BL, :], in_=x0s)
            x0t_sb = small.tile([P, KC, BL], f32, tag="x0t")
            for k in range(KC):
                x0t_ps = misc_ps.tile([P, P], f32, tag="misc")
                nc.tensor.transpose(
                    x0t_ps, x0_nat[:, k * P : (k + 1) * P], identf
                )
                nc.vector.tensor_copy(out=x0t_sb[:, k, :], in_=x0t_ps[:, :BL])

            s1t_sb = small.tile([P, NCH, BL], f32, tag="s1t")
            for n in range(NCH):
                s1_ps = misc_ps.tile([P, BL], f32, tag="misc")
                for k in range(KC):
                    nc.tensor.matmul(
                        s1_ps,
                        lhsT=wa_sb[:, k, n * P : (n + 1) * P],
                        rhs=x0t_sb[:, k, :],
                        start=(k == 0),
                        stop=(k == KC - 1),
                    )
                nc.vector.tensor_copy(out=s1t_sb[:, n, :], in_=s1_ps)

            # ---------------- main loop ----------------
            # o_parts column layout: (n*BL + b)*2 + half
            o_parts = small.tile([P, NCH * BL * 2], f32, tag="o_parts")
            xsums = small.tile([P, 2 * BL * NTB], f32, tag="xsums")

            def issue_stt(bb, n, h):
                """V_a-weighted sum of s_sb[bb] over t-half h of chunk n."""
                scr = scr_pool.tile([P, 2, TBLK], f32, tag="scr")
                idx = (n * BL + bb) * 2 + h
                nc.vector.scalar_tensor_tensor(
                    out=scr,
                    in0=s_tiles[bb][:, n, 2 * h : 2 * h + 2, :],
                    scalar=1.0,
                    in1=va_sb[:, 2 * h : 2 * h + 2, :],
                    op0=mybir.AluOpType.mult,
                    op1=mybir.AluOpType.mult,
                    accum_out=o_parts[:, idx : idx + 1],
                )

            s_tiles = {}
            for b in range(BL):
                s_sb = s_pool.tile([P, NCH, NTB, TBLK], f32, tag="s")
                s_tiles[b] = s_sb
                for tb in range(NTB):
                    it = b * NTB + tb
                    if it < NPRE:
                        nat = prenat[it]
                    else:
                        nat = nat_pool.tile([P, SUB, D], bf16, tag="nat")
                        src = x1s[b, tb * TBLK : (tb + 1) * TBLK, :].rearrange(
                            "(s p) d -> p s d", p=P
                        )
                        nc.gpsimd.dma_start(out=nat, in_=src)  # SWDGE cast

                    # x1 tile -> transposed k-chunks, split 3+1 across two
                    # psum tiles so the copies start early and free them fast
                    xt = xt_pool.tile([P, KC, TBLK], fp8, tag="xt")
                    tpa = tpa_pool.tile([P, 3, TBLK], f32, tag="tpa")
                    for dk in range(3):
                        for s in range(SUB):
                            nc.tensor.matmul(
                                tpa[:, dk, s * P : (s + 1) * P],
                                lhsT=nat[:, s, dk * P : (dk + 1) * P],
                                rhs=ident,
                                start=True,
                                stop=True,
                            )
                    # DVE: chunks 0-2 -> fp8 (+ x1 partial sums via accum)
                    nc.vector.tensor_scalar(
                        out=xt[:, 0:3, :],
                        in0=tpa,
                        scalar1=0.0,
                        scalar2=0.0,
                        op0=mybir.AluOpType.add,
                        op1=mybir.AluOpType.add,
                        accum_out=xsums[:, 2 * it : 2 * it + 1],
                    )
                    tpb = tpb_pool.tile([P, 1, TBLK], f32, tag="tpb")
                    for s in range(SUB):
                        nc.tensor.matmul(
                            tpb[:, 0, s * P : (s + 1) * P],
                            lhsT=nat[:, s, 3 * P : 4 * P],
                            rhs=ident,
                            start=True,
                            stop=True,
                        )
                    # ACT: chunk 3 -> fp8 (+ accum)
                    nc.scalar.activation(
                        out=xt[:, 3, :],
                        in_=tpb[:, 0, :],
                        func=mybir.ActivationFunctionType.Copy,
                        accum_out=xsums[:, 2 * it + 1 : 2 * it + 2],
                    )

                    # interleave V_a reductions (DVE FIFO: after this
                    # iter's copy, so the PE-feeding copy isn't delayed):
                    # half 0 of batch b during (b,2),(b,3); half 1 during
                    # (b+1,0),(b+1,1)
                    if tb >= 2:
                        for j in range(2):
                            issue_stt(b, (tb - 2) * 2 + j, 0)
                    elif b > 0:
                        for j in range(2):
                            issue_stt(b - 1, tb * 2 + j, 1)

                    # main matmul: fp8 DoubleRow, 2 MMs of K=256 per n-chunk
                    for n in range(NCH):
                        mm = mm_pool.tile([P, TBLK], f32, tag="mm")
                        for q in range(2):
                            nc.tensor.matmul(
                                mm,
                                lhsT=wh8[:, 2 * q : 2 * q + 2, n * P : (n + 1) * P],
                                rhs=xt[:, 2 * q : 2 * q + 2, :],
                                start=(q == 0),
                                stop=(q == 1),
                                perf_mode=mybir.MatmulPerfMode.DoubleRow,
                            )
                        nc.scalar.activation(
                            out=s_sb[:, n, tb, :],
                            in_=mm,
                            func=mybir.ActivationFunctionType.Tanh,
                            bias=s1t_sb[:, n, b : b + 1],
                            scale=1.0 / WSCALE,
                        )

            # tail: half-1 reductions of the last batch
            for n in range(NCH):
                issue_stt(BL - 1, n, 1)

            # ---------------- epilogue ----------------
            # merge the half-columns, then transpose o -> [b, d]
            o_sb2 = small.tile([P, NCH * BL], f32, tag="o_sb2")
            nc.vector.reduce_sum(
                out=o_sb2,
                in_=o_parts.rearrange("p (q h) -> p q h", h=2),
                axis=mybir.AxisListType.X,
            )
            o3 = o_sb2.rearrange("p (n b) -> p n b", b=BL)
            ot_ps = misc_ps.tile([BL, D], f32, tag="misc")
            for n in range(NCH):
                nc.tensor.transpose(
                    ot_ps[:, n * P : (n + 1) * P], o3[:, n, :], identf
                )
            ot_sb = small.tile([BL, D], f32, tag="ot_sb")
            nc.vector.tensor_copy(out=ot_sb, in_=ot_ps)

            # sum(x1[b]): reduce per-batch accum columns, then contract d
            xb_sb = small.tile([P, BL], f32, tag="xb_sb")
            nc.vector.reduce_sum(
                out=xb_sb,
                in_=xsums.rearrange("p (b q) -> p b q", q=2 * NTB),
                axis=mybir.AxisListType.X,
            )
            sx1_ps = misc_ps.tile([BL, 1], f32, tag="misc")
            nc.tensor.matmul(sx1_ps, lhsT=xb_sb, rhs=ones_col, start=True, stop=True)
            sx1 = small.tile([BL, 1], f32, tag="sx1")
            nc.vector.tensor_copy(out=sx1, in_=sx1_ps)

            # softmax over D, then scale by sum(x1)
            neg_max = small.tile([BL, 1], f32, tag="neg_max")
            nc.vector.reduce_max(
                out=neg_max, in_=ot_sb, axis=mybir.AxisListType.X, negate=True
            )
            exp_sb = small.tile([BL, D], f32, tag="exp_sb")
            sum_exp = small.tile([BL, 1], f32, tag="sum_exp")
            nc.scalar.activation(
                out=exp_sb,
                in_=ot_sb,
                func=mybir.ActivationFunctionType.Exp,
                bias=neg_max,
                accum_out=sum_exp,
            )
            rec = small.tile([BL, 1], f32, tag="rec")
            nc.vector.reciprocal(out=rec, in_=sum_exp)
            scale = small.tile([BL, 1], f32, tag="scale")
            nc.vector.tensor_mul(out=scale, in0=rec, in1=sx1)
            out_sb = small.tile([BL, D], f32, tag="out_sb")
            nc.vector.tensor_scalar_mul(out=out_sb, in0=exp_sb, scalar1=scale)
            nc.sync.dma_start(out=out, in_=out_sb)

    nc.finalize()
    return nc


def make_in_maps(x0, x1, W_a, W_h, V_a):
    x0 = np.ascontiguousarray(x0, dtype=np.float32)
    x1 = np.ascontiguousarray(x1, dtype=np.float32)
    W_a = np.ascontiguousarray(W_a, dtype=np.float32)
    W_h = np.ascontiguousarray(W_h, dtype=np.float32)
    V_a = np.ascontiguousarray(V_a, dtype=np.float32)
    in_maps = []
    for c in range(NCORES):
        sl = slice(c * BL, (c + 1) * BL)
        in_maps.append(
            {
                "x0s": np.ascontiguousarray(x0[sl]),
                "x1s": np.ascontiguousarray(x1[sl]),
                "W_a": W_a,
                "W_h": W_h,
                "V_a": V_a,
            }
        )
    return in_maps


_NC_CACHE = {}


def _build(mode):
    if mode == "v2":
        return build_nc_v2()
    if mode == "a":
        return build_nc_a()
    return build_nc(mode)


def kernel(x0, x1, W_a, W_h, V_a):
    from concourse.bass_utils import run_bass_kernel_spmd

    mode = "v2"
    nc = _NC_CACHE.get(mode)
    if nc is None:
        nc = _NC_CACHE[mode] = _build(mode)
    in_maps = make_in_maps(x0, x1, W_a, W_h, V_a)
    res = run_bass_kernel_spmd(nc, in_maps, core_ids=list(range(NCORES)))
    return np.concatenate([res.results[c]["out"] for c in range(NCORES)], axis=0)

